# revision 20
# baseline (speedup 1.0000x reference)
"""NeighbourSupport sparse-attention kernel for 8x Trainium2 NeuronCores.

Reference computation (per sample, C=256, Ck=Cv=32, H=W=128):
    k  = relu(conv1x1(x, Wk1, bk1))          # (32, H, W)
    k  = dwconv3x3(k, Wdw, bdw)              # (32, H, W), zero pad
    k  = conv1x1(k, Wk3, bk3)                # (9, H, W)
    w  = softmax(k, axis=0)                  # (9, H, W)
    v  = conv1x1(x, Wv, bv)                  # (32, H, W)
    y[c,p] = sum_j w[j,p] * v[c, p+off_j]    # 3x3 neighbourhood, zero pad
    out = x + conv1x1(y, Wo, bo)             # (256, H, W)

Sharding: pure data parallel, one sample per core (B=8, 8 cores).

Per-core layout (v2, the default): channels on SBUF partitions, pixels on
the free dim, with 4 row-groups x 32 channels packed into the 128
partitions so elementwise ops use all DVE/Pool lanes.  The image is
processed in T=8 row-tiles of R=16 rows; k1/v live in [128, 2, 6, 130]
tiles (6 local rows per group: 4 interior + 1-row halo duplicated
between neighbouring groups/tiles by SBUF->SBUF DMA, plus zero pad
columns) so the depthwise conv and the 3x3 neighbourhood aggregation are
pure shifted-view elementwise ops.  The K=256 input convs run as fp32
matmuls packed 4-per-PSUM-bank via tile_position col groups; everything
downstream (dwconv, logits, softmax, aggregation, out conv operands)
is bf16 (full-rate matmuls, DVE 2x), while the residual add stays fp32
exact.  Softmax over the 9 neighbours: ones-matmul partition reduction,
DVE reciprocal, and one-hot matmuls to broadcast per-pixel weights to
the 32 value channels.  x is streamed from HBM exactly once, out
written once (~33.6 MB/core total HBM traffic).
"""

import numpy as np

C = 256
CK = 32
H = 128
W = 128
R = 16           # rows per tile
T = H // R       # 8 tiles
NCH = 4          # chunks per tile
CR = R // NCH    # 4 rows per chunk
N = CR * W       # 512 pixels per chunk
WP = W + 2       # padded row length (130)

MM_DTYPE = "float32r"   # matmul input dtype view ("float32r" or "float32")

TAPS = [(dy, dx) for dy in (-1, 0, 1) for dx in (-1, 0, 1)]  # jj = 3(dy+1)+(dx+1)


def build_nc(mm_dtype=MM_DTYPE):
    from concourse import bacc
    import concourse.mybir as mybir
    import concourse.tile as tile

    dt = mybir.dt
    f32 = dt.float32
    mmdt = getattr(dt, mm_dtype)
    Alu = mybir.AluOpType
    Act = mybir.ActivationFunctionType

    def mm(ap):
        return ap.bitcast(mmdt) if mm_dtype != "float32" else ap

    nc = bacc.Bacc(None, target_bir_lowering=False, debug=True)

    with tile.TileContext(nc) as tc:
        with tc.tile_pool(name="dram", bufs=1, space="DRAM") as dram:
            x_d = dram.tile([C, H, W], f32, kind="ExternalInput", name="x", uniquify=False)
            out_d = dram.tile([C, H, W], f32, kind="ExternalOutput", name="out", uniquify=False)
            wk1_d = dram.tile([2, 128, CK], f32, kind="ExternalInput", name="wk1T", uniquify=False)
            wv_d = dram.tile([2, 128, CK], f32, kind="ExternalInput", name="wvT", uniquify=False)
            wo_d = dram.tile([CK, C], f32, kind="ExternalInput", name="woT", uniquify=False)
            wk3_d = dram.tile([CK, 9], f32, kind="ExternalInput", name="wk3T", uniquify=False)
            wdw_d = dram.tile([CK, 9], f32, kind="ExternalInput", name="wdw9", uniquify=False)
            bk1_d = dram.tile([CK, 1], f32, kind="ExternalInput", name="bk1c", uniquify=False)
            bv_d = dram.tile([CK, 1], f32, kind="ExternalInput", name="bvc", uniquify=False)
            bdw_d = dram.tile([CK, 1], f32, kind="ExternalInput", name="bdwc", uniquify=False)
            bk3_d = dram.tile([9, 1], f32, kind="ExternalInput", name="bk3c", uniquify=False)
            bo_d = dram.tile([128, 2], f32, kind="ExternalInput", name="boc", uniquify=False)
            ones9_d = dram.tile([9, 1], f32, kind="ExternalInput", name="ones9", uniquify=False)
            ones19_d = dram.tile([1, 9], f32, kind="ExternalInput", name="ones19", uniquify=False)
            bcast_d = dram.tile([9, 288], f32, kind="ExternalInput", name="bcast", uniquify=False)

            with (
                tc.tile_pool(name="consts", bufs=1) as cpool,
                tc.tile_pool(name="xp", bufs=3) as xpool,
                tc.tile_pool(name="kvp", bufs=3) as kvpool,
                tc.tile_pool(name="scr", bufs=3) as scpool,
                tc.tile_pool(name="outp", bufs=3) as outpool,
                tc.tile_pool(name="ps_conv", bufs=2, space="PSUM") as psA,
                tc.tile_pool(name="ps_small", bufs=2, space="PSUM") as psS,
                tc.tile_pool(name="ps_wb", bufs=2, space="PSUM") as psW,
                tc.tile_pool(name="ps_out", bufs=2, space="PSUM") as psO,
            ):
                # ---- constants into SBUF ----
                wk1s = cpool.tile([128, 2, CK], f32, name="wk1s")
                wvs = cpool.tile([128, 2, CK], f32, name="wvs")
                for h in range(2):
                    nc.sync.dma_start(out=wk1s[:, h, :], in_=wk1_d[h])
                    nc.sync.dma_start(out=wvs[:, h, :], in_=wv_d[h])
                wos = cpool.tile([CK, C], f32, name="wos")
                nc.sync.dma_start(out=wos[:], in_=wo_d[:])
                wk3s = cpool.tile([CK, 9], f32, name="wk3s")
                nc.sync.dma_start(out=wk3s[:], in_=wk3_d[:])
                wdws = cpool.tile([CK, 9], f32, name="wdws")
                nc.sync.dma_start(out=wdws[:], in_=wdw_d[:])
                bk1s = cpool.tile([CK, 1], f32, name="bk1s")
                nc.sync.dma_start(out=bk1s[:], in_=bk1_d[:])
                bvs = cpool.tile([CK, 1], f32, name="bvs")
                nc.sync.dma_start(out=bvs[:], in_=bv_d[:])
                bdws = cpool.tile([CK, 1], f32, name="bdws")
                nc.sync.dma_start(out=bdws[:], in_=bdw_d[:])
                bk3s = cpool.tile([9, 1], f32, name="bk3s")
                nc.sync.dma_start(out=bk3s[:], in_=bk3_d[:])
                bos = cpool.tile([128, 2], f32, name="bos")
                nc.sync.dma_start(out=bos[:], in_=bo_d[:])
                ones9s = cpool.tile([9, 1], f32, name="ones9s")
                nc.sync.dma_start(out=ones9s[:], in_=ones9_d[:])
                ones19s = cpool.tile([1, 9], f32, name="ones19s")
                nc.sync.dma_start(out=ones19s[:], in_=ones19_d[:])
                bcasts = cpool.tile([9, 288], f32, name="bcasts")
                nc.sync.dma_start(out=bcasts[:], in_=bcast_d[:])

                xt = [None] * T    # (x_lo, x_hi) per tile
                kvt = [None] * T   # (k1, v) per tile

                def emit_A(t):
                    x_lo = xpool.tile([128, R, W], f32, name="x_lo")
                    x_hi = xpool.tile([128, R, W], f32, name="x_hi")
                    nc.sync.dma_start(out=x_lo[:], in_=x_d[0:128, t * R:(t + 1) * R, :])
                    nc.sync.dma_start(out=x_hi[:], in_=x_d[128:256, t * R:(t + 1) * R, :])
                    k1 = kvpool.tile([CK, R + 2, WP], f32, name="k1")
                    v = kvpool.tile([CK, R + 2, WP], f32, name="v")
                    # zero the left/right pad columns
                    nc.gpsimd.memset(k1[:, :, 0:1], 0.0)
                    nc.gpsimd.memset(k1[:, :, WP - 1:WP], 0.0)
                    nc.gpsimd.memset(v[:, :, 0:1], 0.0)
                    nc.gpsimd.memset(v[:, :, WP - 1:WP], 0.0)
                    for q in range(NCH):
                        xl = x_lo[:, q * CR:(q + 1) * CR, :]
                        xh = x_hi[:, q * CR:(q + 1) * CR, :]
                        kp = psA.tile([CK, CR, W], f32, name="kp", tag="ps_conv")
                        nc.tensor.matmul(kp[:], mm(wk1s[:, 0, :]), mm(xl), start=True, stop=False)
                        nc.tensor.matmul(kp[:], mm(wk1s[:, 1, :]), mm(xh), start=False, stop=True)
                        nc.scalar.activation(k1[:, 1 + q * CR:1 + (q + 1) * CR, 1:1 + W],
                                             kp[:], Act.Relu, bias=bk1s[:, 0:1])
                        vp = psA.tile([CK, CR, W], f32, name="vp", tag="ps_conv")
                        nc.tensor.matmul(vp[:], mm(wvs[:, 0, :]), mm(xl), start=True, stop=False)
                        nc.tensor.matmul(vp[:], mm(wvs[:, 1, :]), mm(xh), start=False, stop=True)
                        nc.scalar.activation(v[:, 1 + q * CR:1 + (q + 1) * CR, 1:1 + W],
                                             vp[:], Act.Identity, bias=bvs[:, 0:1])
                    xt[t] = (x_lo, x_hi)
                    kvt[t] = (k1, v)

                def emit_B(u):
                    k1, v = kvt[u]
                    # fill halo rows (row 0 = image row u*R-1, row R+1 = image row u*R+R)
                    if u > 0:
                        pk1, pv = kvt[u - 1]
                        nc.scalar.copy(k1[:, 0, :], pk1[:, R, :])
                        nc.scalar.copy(v[:, 0, :], pv[:, R, :])
                    else:
                        nc.gpsimd.memset(k1[:, 0, :], 0.0)
                        nc.gpsimd.memset(v[:, 0, :], 0.0)
                    if u < T - 1:
                        nk1, nv = kvt[u + 1]
                        nc.scalar.copy(k1[:, R + 1, :], nk1[:, 1, :])
                        nc.scalar.copy(v[:, R + 1, :], nv[:, 1, :])
                    else:
                        nc.gpsimd.memset(k1[:, R + 1, :], 0.0)
                        nc.gpsimd.memset(v[:, R + 1, :], 0.0)

                    x_lo, x_hi = xt[u]
                    out_lo = outpool.tile([128, R, W], f32, name="out_lo")
                    out_hi = outpool.tile([128, R, W], f32, name="out_hi")

                    for q in range(NCH):
                        r0 = 1 + q * CR  # local row of first output row of chunk

                        # depthwise 3x3 on k1 -> k2
                        k2 = scpool.tile([CK, CR, W], f32, name="k2")
                        for jj, (dy, dx) in enumerate(TAPS):
                            kv_view = k1[:, r0 + dy:r0 + dy + CR, 1 + dx:1 + dx + W]
                            if jj == 0:
                                nc.vector.tensor_scalar(k2[:], kv_view,
                                                        wdws[:, 0:1], bdws[:, 0:1],
                                                        Alu.mult, Alu.add)
                            else:
                                nc.vector.scalar_tensor_tensor(k2[:], kv_view,
                                                               wdws[:, jj:jj + 1], k2[:],
                                                               Alu.mult, Alu.add)

                        # logits -> exp -> sum -> reciprocal -> normalized w9
                        lg = psS.tile([9, CR, W], f32, name="lg", tag="ps_small")
                        nc.tensor.matmul(lg[:], mm(wk3s[:]), mm(k2[:]), start=True, stop=True)
                        e = scpool.tile([9, CR, W], f32, name="e")
                        nc.scalar.activation(e[:], lg[:], Act.Exp, bias=bk3s[:, 0:1])
                        S = psS.tile([1, CR, W], f32, name="S", tag="ps_small")
                        nc.tensor.matmul(S[:], mm(ones9s[:]), mm(e[:]), start=True, stop=True)
                        rc = scpool.tile([1, CR, W], f32, name="rc")
                        nc.vector.reciprocal(rc[:], S[:])
                        r9 = psS.tile([9, CR, W], f32, name="r9", tag="ps_small")
                        nc.tensor.matmul(r9[:], mm(ones19s[:]), mm(rc[:]), start=True, stop=True)
                        w9 = scpool.tile([9, CR, W], f32, name="w9")
                        nc.vector.tensor_tensor(w9[:], e[:], r9[:], Alu.mult)

                        # aggregation: y[c,p] = sum_j w9[j,p] * v[c, p+off_j]
                        y_acc = scpool.tile([CK, CR, W], f32, name="y_acc")
                        for g in range(3):
                            wb = psW.tile([96, CR, W], f32, name="wb", tag="ps_wb")
                            nc.tensor.matmul(wb[:], mm(bcasts[:, 96 * g:96 * (g + 1)]),
                                             mm(w9[:]), start=True, stop=True)
                            for a in range(3):
                                jj = 3 * g + a
                                dy, dx = TAPS[jj]
                                v_view = v[:, r0 + dy:r0 + dy + CR, 1 + dx:1 + dx + W]
                                wbs = wb[32 * a:32 * (a + 1), :, :]
                                if jj == 0:
                                    nc.vector.tensor_tensor(y_acc[:], wbs, v_view, Alu.mult)
                                else:
                                    pr = scpool.tile([CK, CR, W], f32, name="pr")
                                    nc.vector.tensor_tensor(pr[:], wbs, v_view, Alu.mult)
                                    nc.gpsimd.tensor_tensor(y_acc[:], y_acc[:], pr[:], Alu.add)

                        # out conv + bias + residual
                        op_lo = psO.tile([128, CR, W], f32, name="op_lo", tag="ps_out")
                        nc.tensor.matmul(op_lo[:], mm(wos[:, 0:128]), mm(y_acc[:]),
                                         start=True, stop=True)
                        nc.vector.scalar_tensor_tensor(out_lo[:, q * CR:(q + 1) * CR, :],
                                                       op_lo[:], bos[:, 0:1],
                                                       x_lo[:, q * CR:(q + 1) * CR, :],
                                                       Alu.add, Alu.add)
                        op_hi = psO.tile([128, CR, W], f32, name="op_hi", tag="ps_out")
                        nc.tensor.matmul(op_hi[:], mm(wos[:, 128:256]), mm(y_acc[:]),
                                         start=True, stop=True)
                        nc.vector.scalar_tensor_tensor(out_hi[:, q * CR:(q + 1) * CR, :],
                                                       op_hi[:], bos[:, 1:2],
                                                       x_hi[:, q * CR:(q + 1) * CR, :],
                                                       Alu.add, Alu.add)

                    nc.sync.dma_start(out=out_d[0:128, u * R:(u + 1) * R, :], in_=out_lo[:])
                    nc.sync.dma_start(out=out_d[128:256, u * R:(u + 1) * R, :], in_=out_hi[:])

                emit_A(0)
                emit_A(1)
                for t in range(2, T):
                    emit_A(t)
                    emit_B(t - 2)
                emit_B(T - 2)
                emit_B(T - 1)

    nc.compile()
    return nc


def build_nc_v2(mm_dtype=MM_DTYPE, reps=1):
    """Partition-packed variant: 4 row-groups x 32 channels = 128 partitions.

    Each 16-row tile is processed as 4 groups of 4 rows; group a's
    channel-c data lives on partition 32a+c.  Matmuls use tile_position
    col/row groups so the 4 per-group matmuls pack into the PE array and
    one PSUM bank; elementwise ops run on all 128 partitions (4x fewer
    DVE/Pool ops than the unpacked variant).  k1p/vp tiles hold 6 local
    rows per group (1-row halo duplicated between neighbouring groups by
    SBUF->SBUF DMA, cross-tile for group 0/3 edges).
    """
    from concourse import bacc
    import concourse.mybir as mybir
    import concourse.tile as tile

    dt = mybir.dt
    f32 = dt.float32
    bf16 = dt.bfloat16
    Alu = mybir.AluOpType
    Act = mybir.ActivationFunctionType

    nc = bacc.Bacc(None, target_bir_lowering=False, debug=True)

    with tile.TileContext(nc) as tc:
        with tc.tile_pool(name="dram", bufs=1, space="DRAM") as dram:
            x_d = dram.tile([C, H, W], f32, kind="ExternalInput", name="x", uniquify=False)
            out_d = dram.tile([C, H, W], f32, kind="ExternalOutput", name="out", uniquify=False)
            wk1_d = dram.tile([2, 128, CK], f32, kind="ExternalInput", name="wk1T", uniquify=False)
            wv_d = dram.tile([2, 128, CK], f32, kind="ExternalInput", name="wvT", uniquify=False)
            wop_d = dram.tile([128, C], bf16, kind="ExternalInput", name="wop", uniquify=False)
            wk3p_d = dram.tile([128, 9], bf16, kind="ExternalInput", name="wk3p", uniquify=False)
            wdwp_d = dram.tile([128, 9], f32, kind="ExternalInput", name="wdwp", uniquify=False)
            bk1p_d = dram.tile([128, 1], f32, kind="ExternalInput", name="bk1p", uniquify=False)
            bvp_d = dram.tile([128, 1], f32, kind="ExternalInput", name="bvp", uniquify=False)
            bdwp_d = dram.tile([128, 1], f32, kind="ExternalInput", name="bdwp", uniquify=False)
            bk3p_d = dram.tile([128, 1], f32, kind="ExternalInput", name="bk3p", uniquify=False)
            bo_d = dram.tile([128, 2], f32, kind="ExternalInput", name="boc", uniquify=False)
            ones_d = dram.tile([128, 32], bf16, kind="ExternalInput", name="ones32", uniquify=False)
            bc2_d = dram.tile([128, 288], bf16, kind="ExternalInput", name="bc2", uniquify=False)

            with (
                tc.tile_pool(name="consts", bufs=1) as cpool,
                tc.tile_pool(name="xp", bufs=3) as xpool,
                tc.tile_pool(name="kvp", bufs=3) as kvpool,
                tc.tile_pool(name="scr", bufs=3) as scpool,
                tc.tile_pool(name="outp", bufs=3) as outpool,
                tc.tile_pool(name="ps_conv", bufs=2, space="PSUM") as psA,
                tc.tile_pool(name="ps_small", bufs=2, space="PSUM") as psS,
                tc.tile_pool(name="ps_wb", bufs=2, space="PSUM") as psW,
                tc.tile_pool(name="ps_out", bufs=2, space="PSUM") as psO,
            ):
                wk1s = cpool.tile([128, 2, CK], f32, name="wk1s")
                wvs = cpool.tile([128, 2, CK], f32, name="wvs")
                for h in range(2):
                    nc.sync.dma_start(out=wk1s[:, h, :], in_=wk1_d[h])
                    nc.sync.dma_start(out=wvs[:, h, :], in_=wv_d[h])
                wops = cpool.tile([128, C], bf16, name="wops")
                nc.sync.dma_start(out=wops[:], in_=wop_d[:])
                wk3s = cpool.tile([128, 9], bf16, name="wk3s")
                nc.sync.dma_start(out=wk3s[:], in_=wk3p_d[:])
                wdws = cpool.tile([128, 9], f32, name="wdws")
                nc.sync.dma_start(out=wdws[:], in_=wdwp_d[:])
                bk1s = cpool.tile([128, 1], f32, name="bk1s")
                nc.sync.dma_start(out=bk1s[:], in_=bk1p_d[:])
                bvs = cpool.tile([128, 1], f32, name="bvs")
                nc.sync.dma_start(out=bvs[:], in_=bvp_d[:])
                bdws = cpool.tile([128, 1], f32, name="bdws")
                nc.sync.dma_start(out=bdws[:], in_=bdwp_d[:])
                bk3s = cpool.tile([128, 1], f32, name="bk3s")
                nc.sync.dma_start(out=bk3s[:], in_=bk3p_d[:])
                bos = cpool.tile([128, 2], f32, name="bos")
                nc.sync.dma_start(out=bos[:], in_=bo_d[:])
                oness = cpool.tile([128, 32], bf16, name="oness")
                nc.sync.dma_start(out=oness[:], in_=ones_d[:])
                bc2s = cpool.tile([128, 288], bf16, name="bc2s")
                nc.sync.dma_start(out=bc2s[:], in_=bc2_d[:])

                xt = [None] * T
                kvt = [None] * T

                def emit_A(t):
                    x_lo = xpool.tile([128, R, W], f32, name="x_lo")
                    x_hi = xpool.tile([128, R, W], f32, name="x_hi")
                    nc.sync.dma_start(out=x_lo[:], in_=x_d[0:128, t * R:(t + 1) * R, :])
                    nc.sync.dma_start(out=x_hi[:], in_=x_d[128:256, t * R:(t + 1) * R, :])
                    kv = kvpool.tile([128, 2, 6, WP], bf16, name="kv")
                    k1p = kv[:, 0]
                    vp = kv[:, 1]
                    nc.gpsimd.memset(kv[:, :, :, 0:1], 0.0)
                    nc.gpsimd.memset(kv[:, :, :, WP - 1:WP], 0.0)
                    kp = psA.tile([128, CR, W], f32, name="kp", tag="ps_conv")
                    vpp = psA.tile([128, CR, W], f32, name="vpp", tag="ps_conv")
                    for a in range(4):
                        xl = x_lo[:, 4 * a:4 * a + 4, :]
                        xh = x_hi[:, 4 * a:4 * a + 4, :]
                        po = kp[32 * a:32 * (a + 1), :, :]
                        nc.tensor.matmul(po, wk1s[:, 0, :], xl,
                                         start=True, stop=False, tile_position=(0, 32 * a))
                        nc.tensor.matmul(po, wk1s[:, 1, :], xh,
                                         start=False, stop=True, tile_position=(0, 32 * a))
                        po = vpp[32 * a:32 * (a + 1), :, :]
                        nc.tensor.matmul(po, wvs[:, 0, :], xl,
                                         start=True, stop=False, tile_position=(0, 32 * a))
                        nc.tensor.matmul(po, wvs[:, 1, :], xh,
                                         start=False, stop=True, tile_position=(0, 32 * a))
                    nc.scalar.activation(k1p[:, 1:5, 1:1 + W], kp[:], Act.Relu,
                                         bias=bk1s[:, 0:1])
                    nc.scalar.activation(vp[:, 1:5, 1:1 + W], vpp[:], Act.Identity,
                                         bias=bvs[:, 0:1])
                    # duplicate halo rows between neighbouring groups (intra-tile)
                    nc.sync.dma_start(out=kv[32:128, :, 0, :], in_=kv[0:96, :, 4, :])
                    nc.sync.dma_start(out=kv[0:96, :, 5, :], in_=kv[32:128, :, 1, :])
                    xt[t] = (x_lo, x_hi)
                    kvt[t] = kv

                def emit_B(u):
                    kv = kvt[u]
                    k1p = kv[:, 0]
                    vp = kv[:, 1]
                    # cross-tile halo rows for group 0 (top) and group 3 (bottom)
                    if u > 0:
                        nc.sync.dma_start(out=kv[0:32, :, 0, :],
                                          in_=kvt[u - 1][96:128, :, 4, :])
                    else:
                        nc.gpsimd.memset(kv[0:32, :, 0, :], 0.0)
                    if u < T - 1:
                        nc.sync.dma_start(out=kv[96:128, :, 5, :],
                                          in_=kvt[u + 1][0:32, :, 1, :])
                    else:
                        nc.gpsimd.memset(kv[96:128, :, 5, :], 0.0)

                    x_lo, x_hi = xt[u]
                    out_lo = outpool.tile([128, R, W], f32, name="out_lo")
                    out_hi = outpool.tile([128, R, W], f32, name="out_hi")

                    def kview(tt, jj):
                        dy, dx = TAPS[jj]
                        return tt[:, 1 + dy:5 + dy, 1 + dx:1 + dx + W]

                    # depthwise 3x3: 9 taps chained on DVE (bf16, 2x mode)
                    k2m = scpool.tile([128, CR, W], bf16, name="k2m")
                    with nc.allow_low_precision(reason="bf16 dwconv accumulation"):
                        nc.vector.tensor_scalar(k2m[:], kview(k1p, 0), wdws[:, 0:1],
                                                bdws[:, 0:1], Alu.mult, Alu.add)
                        for jj in range(1, 9):
                            nc.vector.scalar_tensor_tensor(k2m[:], kview(k1p, jj),
                                                           wdws[:, jj:jj + 1], k2m[:],
                                                           Alu.mult, Alu.add)

                    # logits / exp / sum / recip per group (tile-packed matmuls)
                    lg = psS.tile([128, CR, W], f32, name="lg", tag="ps_small")
                    for a in range(4):
                        nc.tensor.matmul(lg[32 * a:32 * a + 9, :, :],
                                         wk3s[32 * a:32 * (a + 1), :],
                                         k2m[32 * a:32 * (a + 1), :, :],
                                         start=True, stop=True,
                                         tile_position=(32 * a, 32 * a))
                    e = scpool.tile([128, CR, W], bf16, name="e")
                    for a in range(4):
                        nc.scalar.activation(e[32 * a:32 * a + 9, :, :],
                                             lg[32 * a:32 * a + 9, :, :], Act.Exp,
                                             bias=bk3s[32 * a:32 * a + 9, 0:1])
                    S = psS.tile([128, CR, W], f32, name="S", tag="ps_small")
                    for a in range(4):
                        nc.tensor.matmul(S[32 * a:32 * a + 1, :, :],
                                         oness[32 * a:32 * a + 9, 0:1],
                                         e[32 * a:32 * a + 9, :, :],
                                         start=True, stop=True,
                                         tile_position=(32 * a, 32 * a))
                    rc = scpool.tile([128, CR, W], bf16, name="rc")
                    with nc.allow_low_precision(reason="bf16 softmax recip"):
                        for a in range(4):
                            nc.vector.reciprocal(rc[32 * a:32 * a + 1, :, :],
                                                 S[32 * a:32 * a + 1, :, :])
                    rb = psS.tile([128, CR, W], f32, name="rb", tag="ps_small")
                    for a in range(4):
                        nc.tensor.matmul(rb[32 * a:32 * (a + 1), :, :],
                                         oness[32 * a:32 * a + 1, 0:32],
                                         rc[32 * a:32 * a + 1, :, :],
                                         start=True, stop=True,
                                         tile_position=(32 * a, 32 * a))
                    # aggregation with unnormalized weights, normalize at the end;
                    # products on DVE, pairwise tree-adds on Pool
                    prods = []
                    for jj in range(9):
                        wb = psW.tile([128, CR, W], f32, name="wb", tag="ps_wb")
                        for a in range(4):
                            nc.tensor.matmul(wb[32 * a:32 * (a + 1), :, :],
                                             bc2s[32 * a:32 * a + 9, 32 * jj:32 * (jj + 1)],
                                             e[32 * a:32 * a + 9, :, :],
                                             start=True, stop=True,
                                             tile_position=(32 * a, 32 * a))
                        vv = kview(vp, jj)
                        pr = scpool.tile([128, CR, W], f32, name="pr", bufs=6)
                        nc.vector.tensor_tensor(pr[:], wb[:], vv, Alu.mult)
                        prods.append(pr)
                        if jj % 2 == 1:   # fold pairs as they arrive
                            nc.gpsimd.tensor_tensor(prods[jj - 1][:], prods[jj - 1][:],
                                                    prods[jj][:], Alu.add)
                    s0, s1, s2, s3, p8 = prods[0], prods[2], prods[4], prods[6], prods[8]
                    nc.gpsimd.tensor_tensor(s3[:], s3[:], p8[:], Alu.add)
                    nc.gpsimd.tensor_tensor(s0[:], s0[:], s1[:], Alu.add)
                    nc.gpsimd.tensor_tensor(s2[:], s2[:], s3[:], Alu.add)
                    nc.gpsimd.tensor_tensor(s0[:], s0[:], s2[:], Alu.add)
                    y_bf = scpool.tile([128, CR, W], bf16, name="y_bf")
                    nc.vector.tensor_tensor(y_bf[:], s0[:], rb[:], Alu.mult)

                    # out conv + bias via ACT, residual add on Pool
                    for half, (xh, outh) in enumerate(((x_lo, out_lo), (x_hi, out_hi))):
                        for a in range(4):
                            op = psO.tile([128, CR, W], f32, name="op", tag="ps_out")
                            nc.tensor.matmul(op[:],
                                             wops[32 * a:32 * (a + 1), 128 * half:128 * (half + 1)],
                                             y_bf[32 * a:32 * (a + 1), :, :],
                                             start=True, stop=True,
                                             tile_position=(32 * a, 0))
                            ov = outh[:, 4 * a:4 * a + 4, :]
                            nc.scalar.activation(ov, op[:], Act.Identity,
                                                 bias=bos[:, half:half + 1])
                            nc.gpsimd.tensor_tensor(ov, ov, xh[:, 4 * a:4 * a + 4, :],
                                                    Alu.add)

                    nc.sync.dma_start(out=out_d[0:128, u * R:(u + 1) * R, :], in_=out_lo[:])
                    nc.sync.dma_start(out=out_d[128:256, u * R:(u + 1) * R, :], in_=out_hi[:])

                def emit_all():
                    for i in range(T):
                        xt[i] = None
                        kvt[i] = None
                    emit_A(0)
                    emit_A(1)
                    for t in range(2, T):
                        emit_A(t)
                        emit_B(t - 2)
                    emit_B(T - 2)
                    emit_B(T - 1)

                if reps > 1:
                    with tc.For_i(0, reps, 1):
                        emit_all()
                else:
                    emit_all()

    nc.compile()
    return nc


def build_nc_v3():
    """v3: blockdiag weights instead of tile_position packing, f32r input
    convs, dwconv+Wk3 fused into 9 accumulating matmuls, packed exp,
    reciprocal_approx_fast, residual via identity matmul into PSUM.

    Layout identical to v2: 4 row-groups x 32 channels on 128 partitions,
    T=8 tiles of R=16 rows, kv tiles [128, 2, 6, 130] bf16 with halo rows.
    Group-a quantities that are per-pixel scalars live at partitions 9a+o
    (logits/e, o<9) or a (sums), via block-diagonal lhsT weights.
    """
    from concourse import bacc
    import concourse.mybir as mybir
    import concourse.tile as tile

    dt = mybir.dt
    f32 = dt.float32
    f32r = dt.float32r
    bf16 = dt.bfloat16
    f8 = dt.float8e4
    DR = mybir.MatmulPerfMode.DoubleRow
    Alu = mybir.AluOpType
    Act = mybir.ActivationFunctionType

    def r(ap):
        return ap.bitcast(f32r)

    nc = bacc.Bacc(None, target_bir_lowering=False, debug=True)

    with tile.TileContext(nc) as tc:
        with tc.tile_pool(name="dram", bufs=1, space="DRAM") as dram:
            x_d = dram.tile([C, H, W], f32, kind="ExternalInput", name="x", uniquify=False)
            out_d = dram.tile([C, H, W], f32, kind="ExternalOutput", name="out", uniquify=False)
            wk1_d = dram.tile([2, 128, CK], bf16, kind="ExternalInput", name="wk1T", uniquify=False)
            wv_d = dram.tile([2, 128, CK], bf16, kind="ExternalInput", name="wvT", uniquify=False)
            wop_d = dram.tile([128, C], bf16, kind="ExternalInput", name="wop", uniquify=False)
            mf_d = dram.tile([128, 9, 36], bf16, kind="ExternalInput", name="mf", uniquify=False)
            sones_d = dram.tile([36, 4], bf16, kind="ExternalInput", name="sones", uniquify=False)
            bk1p_d = dram.tile([128, 1], f32, kind="ExternalInput", name="bk1p", uniquify=False)
            bvp_d = dram.tile([128, 1], f32, kind="ExternalInput", name="bvp", uniquify=False)
            bk3f_d = dram.tile([36, 1], f32, kind="ExternalInput", name="bk3f", uniquify=False)
            bo_d = dram.tile([128, 2], f32, kind="ExternalInput", name="boc", uniquify=False)

            with (
                tc.tile_pool(name="consts", bufs=1) as cpool,
                tc.tile_pool(name="xbp", bufs=5) as xbpool,
                tc.tile_pool(name="wbp", bufs=18) as wbpool,
                tc.tile_pool(name="kvp", bufs=4) as kvpool,
                tc.tile_pool(name="scr", bufs=3) as scpool,
                tc.tile_pool(name="outp", bufs=3) as outpool,
                tc.tile_pool(name="ps_conv", bufs=3, space="PSUM") as psA,
                tc.tile_pool(name="ps_small", bufs=2, space="PSUM") as psS,
                tc.tile_pool(name="ps_out", bufs=3, space="PSUM") as psO,

            ):
                wk1s = cpool.tile([128, 2, CK], bf16, name="wk1s")
                wvs = cpool.tile([128, 2, CK], bf16, name="wvs")
                for h in range(2):
                    nc.sync.dma_start(out=wk1s[:, h, :], in_=wk1_d[h])
                    nc.sync.dma_start(out=wvs[:, h, :], in_=wv_d[h])
                wops = cpool.tile([128, C], bf16, name="wops")
                nc.sync.dma_start(out=wops[:], in_=wop_d[:])
                mfs = cpool.tile([128, 9, 36], bf16, name="mfs")
                nc.sync.dma_start(out=mfs[:], in_=mf_d[:])
                soness = cpool.tile([36, 4], bf16, name="soness")
                nc.sync.dma_start(out=soness[:], in_=sones_d[:])
                bk1s = cpool.tile([128, 1], f32, name="bk1s")
                nc.sync.dma_start(out=bk1s[:], in_=bk1p_d[:])
                bvs = cpool.tile([128, 1], f32, name="bvs")
                nc.sync.dma_start(out=bvs[:], in_=bvp_d[:])
                bk3s = cpool.tile([36, 1], f32, name="bk3s")
                nc.sync.dma_start(out=bk3s[:], in_=bk3f_d[:])
                bos = cpool.tile([128, 2], f32, name="bos")
                nc.sync.dma_start(out=bos[:], in_=bo_d[:])

                xt = [None] * T
                kvt = [None] * T
                et = [None] * T

                def emit_A(t):
                    xb_lo = xbpool.tile([128, R, W], bf16, name="xb_lo")
                    xb_hi = xbpool.tile([128, R, W], bf16, name="xb_hi")
                    nc.gpsimd.dma_start(out=xb_lo[:], in_=x_d[0:128, t * R:(t + 1) * R, :])
                    nc.gpsimd.dma_start(out=xb_hi[:], in_=x_d[128:256, t * R:(t + 1) * R, :])
                    kv = kvpool.tile([128, 2, 6, WP], bf16, name="kv")
                    k1p = kv[:, 0]
                    vp = kv[:, 1]
                    nc.gpsimd.memset(kv[:, :, :, 0:1], 0.0)
                    nc.gpsimd.memset(kv[:, :, :, WP - 1:WP], 0.0)
                    kp = psA.tile([128, CR, W], f32, name="kp", tag="ps_conv")
                    vpp = psA.tile([128, CR, W], f32, name="vpp", tag="ps_conv")
                    for a in range(4):
                        xl = xb_lo[:, 4 * a:4 * a + 4, :]
                        xh = xb_hi[:, 4 * a:4 * a + 4, :]
                        po = kp[32 * a:32 * (a + 1), :, :]
                        nc.tensor.matmul(po, wk1s[:, 0, :], xl,
                                         start=True, stop=False, tile_position=(0, 32 * a))
                        nc.tensor.matmul(po, wk1s[:, 1, :], xh,
                                         start=False, stop=True, tile_position=(0, 32 * a))
                        po = vpp[32 * a:32 * (a + 1), :, :]
                        nc.tensor.matmul(po, wvs[:, 0, :], xl,
                                         start=True, stop=False, tile_position=(0, 32 * a))
                        nc.tensor.matmul(po, wvs[:, 1, :], xh,
                                         start=False, stop=True, tile_position=(0, 32 * a))
                    nc.scalar.activation(k1p[:, 1:5, 1:1 + W], kp[:], Act.Relu,
                                         bias=bk1s[:, 0:1])
                    nc.scalar.activation(vp[:, 1:5, 1:1 + W], vpp[:], Act.Identity,
                                         bias=bvs[:, 0:1])
                    nc.gpsimd.dma_start(out=kv[32:128, :, 0, :], in_=kv[0:96, :, 4, :])
                    nc.gpsimd.dma_start(out=kv[0:96, :, 5, :], in_=kv[32:128, :, 1, :])
                    xt[t] = (xb_lo, xb_hi)
                    kvt[t] = kv

                def emit_B1(u):
                    """Softmax pipe: halos, fused dwconv+Wk3 logits, exp,
                    sum, reciprocal, reciprocal broadcast."""
                    kv = kvt[u]
                    k1p = kv[:, 0]
                    if u > 0:
                        nc.gpsimd.dma_start(out=kv[0:32, :, 0, :],
                                            in_=kvt[u - 1][96:128, :, 4, :])
                    else:
                        nc.gpsimd.memset(kv[0:32, :, 0, :], 0.0)
                    if u < T - 1:
                        nc.gpsimd.dma_start(out=kv[96:128, :, 5, :],
                                            in_=kvt[u + 1][0:32, :, 1, :])
                    else:
                        nc.gpsimd.memset(kv[96:128, :, 5, :], 0.0)

                    def kview(tt, jj):
                        dy, dx = TAPS[jj]
                        return tt[:, 1 + dy:5 + dy, 1 + dx:1 + dx + W]

                    lg = psS.tile([36, CR, W], f32, name="lg", tag="ps_small")
                    for jj in range(9):
                        nc.tensor.matmul(lg[:], mfs[:, jj, :], kview(k1p, jj),
                                         start=(jj == 0), stop=(jj == 8))
                    e = scpool.tile([36, CR, W], bf16, name="e")
                    with nc.allow_low_precision(reason="bf16 softmax weights"):
                        nc.scalar.activation(e[:], lg[:], Act.Exp, bias=bk3s[:, 0:1])
                    S = psS.tile([4, CR, W], f32, name="S", tag="ps_small")
                    nc.tensor.matmul(S[:], soness[:], e[:], start=True, stop=True)
                    rc = scpool.tile([4, CR, W], f32, name="rc")
                    with nc.allow_low_precision(reason="approx reciprocal"):
                        nc.vector.reciprocal_approx_fast(out=rc[:], in_=S[:])
                    rc_bf = scpool.tile([4, CR, W], bf16, name="rc_bf")
                    with nc.allow_low_precision(reason="bf16 softmax recip"):
                        nc.scalar.copy(rc_bf[:], rc[:])
                    rb_sb = scpool.tile([128, CR, W], bf16, name="rb_sb")
                    nc.gpsimd.dma_start(
                        out=rb_sb[:],
                        in_=rc_bf[:].unsqueeze(1).broadcast_to((4, 32, CR, W)))
                    wbs_list = []
                    for jj in range(9):
                        wbs = wbpool.tile([128, CR, W], bf16, name="wbs")
                        nc.sync.dma_start(
                            out=wbs[:],
                            in_=e[jj::9, :, :].unsqueeze(1).broadcast_to((4, 32, CR, W)))
                        wbs_list.append(wbs)
                    et[u] = (wbs_list, rb_sb)

                def emit_B2(u):
                    """Aggregation + out conv + residual + store."""
                    kv = kvt[u]
                    vp = kv[:, 1]
                    wbs_list, rb = et[u]
                    x_lo, x_hi = xt[u]
                    out_lo = outpool.tile([128, R, W], f32, name="out_lo")
                    out_hi = outpool.tile([128, R, W], f32, name="out_hi")

                    def kview(tt, jj):
                        dy, dx = TAPS[jj]
                        return tt[:, 1 + dy:5 + dy, 1 + dx:1 + dx + W]

                    prods = []
                    for jj in range(9):
                        vv = kview(vp, jj)
                        pr = scpool.tile([128, CR, W], bf16, name="pr", bufs=6)
                        with nc.allow_low_precision(reason="bf16 aggregation"):
                            nc.vector.tensor_tensor(pr[:], wbs_list[jj][:], vv, Alu.mult)
                        prods.append(pr)
                        if jj % 2 == 1:
                            nc.vector.tensor_tensor(prods[jj - 1][:], prods[jj - 1][:],
                                                    prods[jj][:], Alu.add)
                    s0, s1, s2, s3, p8 = prods[0], prods[2], prods[4], prods[6], prods[8]
                    nc.gpsimd.tensor_tensor(s3[:], s3[:], p8[:], Alu.add)
                    nc.gpsimd.tensor_tensor(s0[:], s0[:], s1[:], Alu.add)
                    nc.gpsimd.tensor_tensor(s2[:], s2[:], s3[:], Alu.add)
                    nc.gpsimd.tensor_tensor(s0[:], s0[:], s2[:], Alu.add)
                    y_bf = scpool.tile([128, CR, W], bf16, name="y_bf")
                    with nc.allow_low_precision(reason="bf16 aggregation"):
                        nc.vector.tensor_tensor(y_bf[:], s0[:], rb[:], Alu.mult)

                    for half, (xh, outh) in enumerate(((x_lo, out_lo), (x_hi, out_hi))):
                        for a in range(4):
                            op = psO.tile([128, CR, W], f32, name="op", tag="ps_out")
                            nc.tensor.matmul(op[:],
                                             wops[32 * a:32 * (a + 1), 128 * half:128 * (half + 1)],
                                             y_bf[32 * a:32 * (a + 1), :, :],
                                             start=True, stop=True,
                                             tile_position=(32 * a, 0))
                            ov = outh[:, 4 * a:4 * a + 4, :]
                            if half == 0:
                                nc.vector.scalar_tensor_tensor(ov, op[:],
                                                               bos[:, 0:1],
                                                               xh[:, 4 * a:4 * a + 4, :],
                                                               Alu.add, Alu.add)
                            else:
                                nc.scalar.activation(ov, op[:], Act.Identity,
                                                     bias=bos[:, 1:2])
                                nc.gpsimd.tensor_tensor(ov, ov,
                                                        xh[:, 4 * a:4 * a + 4, :],
                                                        Alu.add)

                    nc.scalar.dma_start(out=out_d[0:128, u * R:(u + 1) * R, :], in_=out_lo[:])
                    nc.scalar.dma_start(out=out_d[128:256, u * R:(u + 1) * R, :], in_=out_hi[:])

                emit_A(0)
                emit_A(1)
                emit_B1(0)
                emit_A(2)
                for t in range(3, T):
                    emit_A(t)
                    emit_B2(t - 3)
                    emit_B1(t - 2)
                emit_B1(T - 2)
                emit_B2(T - 3)
                emit_B1(T - 1)
                emit_B2(T - 2)
                emit_B2(T - 1)

    nc.compile()
    return nc


def make_const_inputs_v3(Wk1, bk1, Wdw, bdw, Wk3, bk3, Wv, bv, Wo, bo):
    import ml_dtypes
    f = np.float32
    bf = ml_dtypes.bfloat16
    f8 = ml_dtypes.float8_e4m3
    Wdw9 = Wdw.reshape(CK, 9).astype(f)       # [c, j]
    mf = np.zeros((128, 9, 36), f)
    for a in range(4):
        for c in range(CK):
            for j in range(9):
                # lg[9a+o] += sum_c Wk3[o,c]*Wdw[c,j] * k1[32a+c, p+delta_j]
                mf[32 * a + c, j, 9 * a:9 * a + 9] = Wk3[:, c] * Wdw9[c, j]
    bk3f = np.zeros((36, 1), f)
    for a in range(4):
        bk3f[9 * a:9 * a + 9, 0] = bk3 + Wk3 @ bdw
    sones = np.zeros((36, 4), f)
    for a in range(4):
        sones[9 * a:9 * a + 9, a] = 1.0
    return {
        "wk1T": np.ascontiguousarray(Wk1.T.reshape(2, 128, CK), f).astype(bf),
        "wvT": np.ascontiguousarray(Wv.T.reshape(2, 128, CK), f).astype(bf),
        "wop": np.ascontiguousarray(np.tile(Wo.T, (4, 1))).astype(bf),
        "mf": mf.astype(bf),
        "sones": sones.astype(bf),
        "bk1p": np.ascontiguousarray(np.tile(bk1.reshape(CK, 1), (4, 1)), f),
        "bvp": np.ascontiguousarray(np.tile(bv.reshape(CK, 1), (4, 1)), f),
        "bk3f": bk3f,
        "boc": np.ascontiguousarray(bo.reshape(2, 128).T, f),
    }


def make_const_inputs_v2(Wk1, bk1, Wdw, bdw, Wk3, bk3, Wv, bv, Wo, bo):
    import ml_dtypes
    f = np.float32
    bf = ml_dtypes.bfloat16
    bc2 = np.zeros((128, 288), bf)
    for a in range(4):
        for j in range(9):
            bc2[32 * a + j, 32 * j:32 * (j + 1)] = 1.0
    bk3p = np.zeros((128, 1), f)
    for a in range(4):
        bk3p[32 * a:32 * a + 9, 0] = bk3
    return {
        "wk1T": np.ascontiguousarray(Wk1.T.reshape(2, 128, CK), f),
        "wvT": np.ascontiguousarray(Wv.T.reshape(2, 128, CK), f),
        "wop": np.ascontiguousarray(np.tile(Wo.T, (4, 1))).astype(bf),
        "wk3p": np.ascontiguousarray(np.tile(Wk3.T, (4, 1))).astype(bf),
        "wdwp": np.ascontiguousarray(np.tile(Wdw.reshape(CK, 9), (4, 1)), f),
        "bk1p": np.ascontiguousarray(np.tile(bk1.reshape(CK, 1), (4, 1)), f),
        "bvp": np.ascontiguousarray(np.tile(bv.reshape(CK, 1), (4, 1)), f),
        "bdwp": np.ascontiguousarray(np.tile(bdw.reshape(CK, 1), (4, 1)), f),
        "bk3p": bk3p,
        "boc": np.ascontiguousarray(bo.reshape(2, 128).T, f),
        "ones32": np.ones((128, 32), bf),
        "bc2": bc2,
    }


def make_const_inputs(Wk1, bk1, Wdw, bdw, Wk3, bk3, Wv, bv, Wo, bo):
    f = np.float32
    bcast = np.zeros((9, 288), f)
    for j in range(9):
        g, a = divmod(j, 3)
        bcast[j, 96 * g + 32 * a:96 * g + 32 * (a + 1)] = 1.0
    return {
        "wk1T": np.ascontiguousarray(Wk1.T.reshape(2, 128, CK), f),
        "wvT": np.ascontiguousarray(Wv.T.reshape(2, 128, CK), f),
        "woT": np.ascontiguousarray(Wo.T, f),
        "wk3T": np.ascontiguousarray(Wk3.T, f),
        "wdw9": np.ascontiguousarray(Wdw.reshape(CK, 9), f),
        "bk1c": np.ascontiguousarray(bk1.reshape(CK, 1), f),
        "bvc": np.ascontiguousarray(bv.reshape(CK, 1), f),
        "bdwc": np.ascontiguousarray(bdw.reshape(CK, 1), f),
        "bk3c": np.ascontiguousarray(bk3.reshape(9, 1), f),
        "boc": np.ascontiguousarray(bo.reshape(2, 128).T, f),
        "ones9": np.ones((9, 1), f),
        "ones19": np.ones((1, 9), f),
        "bcast": bcast,
    }


VERSION = 3

_NC_CACHE = {}


def build():
    if VERSION == 3:
        return build_nc_v3()
    return build_nc_v2(MM_DTYPE) if VERSION == 2 else build_nc(MM_DTYPE)


def consts(**kw):
    if VERSION == 3:
        return make_const_inputs_v3(**kw)
    fn = make_const_inputs_v2 if VERSION == 2 else make_const_inputs
    return fn(**kw)


def _get_nc():
    key = (VERSION, MM_DTYPE)
    if key not in _NC_CACHE:
        _NC_CACHE[key] = build()
    return _NC_CACHE[key]


LAST_RESULT = None


def kernel(x, Wk1, bk1, Wdw, bdw, Wk3, bk3, Wv, bv, Wo, bo):
    global LAST_RESULT
    from concourse.bass_utils import run_bass_kernel_spmd

    x = np.asarray(x, np.float32)
    B = x.shape[0]
    assert B == 8 and x.shape[1:] == (C, H, W)
    cs = consts(Wk1=np.asarray(Wk1), bk1=np.asarray(bk1), Wdw=np.asarray(Wdw),
                bdw=np.asarray(bdw), Wk3=np.asarray(Wk3), bk3=np.asarray(bk3),
                Wv=np.asarray(Wv), bv=np.asarray(bv), Wo=np.asarray(Wo),
                bo=np.asarray(bo))
    nc = _get_nc()
    in_maps = [dict(cs, x=np.ascontiguousarray(x[i])) for i in range(B)]
    res = run_bass_kernel_spmd(nc, in_maps, list(range(B)))
    LAST_RESULT = res
    return np.stack([res.results[i]["out"] for i in range(B)], axis=0)



# revision 22
# speedup vs baseline: 1.1444x; 1.1444x over previous
"""NeighbourSupport sparse-attention kernel for 8x Trainium2 NeuronCores.

Reference computation (per sample, C=256, Ck=Cv=32, H=W=128):
    k  = relu(conv1x1(x, Wk1, bk1))          # (32, H, W)
    k  = dwconv3x3(k, Wdw, bdw)              # (32, H, W), zero pad
    k  = conv1x1(k, Wk3, bk3)                # (9, H, W)
    w  = softmax(k, axis=0)                  # (9, H, W)
    v  = conv1x1(x, Wv, bv)                  # (32, H, W)
    y[c,p] = sum_j w[j,p] * v[c, p+off_j]    # 3x3 neighbourhood, zero pad
    out = x + conv1x1(y, Wo, bo)             # (256, H, W)

Sharding: pure data parallel, one sample per core (B=8, 8 cores).

Per-core layout (v2, the default): channels on SBUF partitions, pixels on
the free dim, with 4 row-groups x 32 channels packed into the 128
partitions so elementwise ops use all DVE/Pool lanes.  The image is
processed in T=8 row-tiles of R=16 rows; k1/v live in [128, 2, 6, 130]
tiles (6 local rows per group: 4 interior + 1-row halo duplicated
between neighbouring groups/tiles by SBUF->SBUF DMA, plus zero pad
columns) so the depthwise conv and the 3x3 neighbourhood aggregation are
pure shifted-view elementwise ops.  The K=256 input convs run as fp32
matmuls packed 4-per-PSUM-bank via tile_position col groups; everything
downstream (dwconv, logits, softmax, aggregation, out conv operands)
is bf16 (full-rate matmuls, DVE 2x), while the residual add stays fp32
exact.  Softmax over the 9 neighbours: ones-matmul partition reduction,
DVE reciprocal, and one-hot matmuls to broadcast per-pixel weights to
the 32 value channels.  x is streamed from HBM exactly once, out
written once (~33.6 MB/core total HBM traffic).
"""

import numpy as np

C = 256
CK = 32
H = 128
W = 128
R = 16           # rows per tile
T = H // R       # 8 tiles
NCH = 4          # chunks per tile
CR = R // NCH    # 4 rows per chunk
N = CR * W       # 512 pixels per chunk
WP = W + 2       # padded row length (130)

MM_DTYPE = "float32r"   # matmul input dtype view ("float32r" or "float32")

TAPS = [(dy, dx) for dy in (-1, 0, 1) for dx in (-1, 0, 1)]  # jj = 3(dy+1)+(dx+1)


def build_nc(mm_dtype=MM_DTYPE):
    from concourse import bacc
    import concourse.mybir as mybir
    import concourse.tile as tile

    dt = mybir.dt
    f32 = dt.float32
    mmdt = getattr(dt, mm_dtype)
    Alu = mybir.AluOpType
    Act = mybir.ActivationFunctionType

    def mm(ap):
        return ap.bitcast(mmdt) if mm_dtype != "float32" else ap

    nc = bacc.Bacc(None, target_bir_lowering=False, debug=True)

    with tile.TileContext(nc) as tc:
        with tc.tile_pool(name="dram", bufs=1, space="DRAM") as dram:
            x_d = dram.tile([C, H, W], f32, kind="ExternalInput", name="x", uniquify=False)
            out_d = dram.tile([C, H, W], f32, kind="ExternalOutput", name="out", uniquify=False)
            wk1_d = dram.tile([2, 128, CK], f32, kind="ExternalInput", name="wk1T", uniquify=False)
            wv_d = dram.tile([2, 128, CK], f32, kind="ExternalInput", name="wvT", uniquify=False)
            wo_d = dram.tile([CK, C], f32, kind="ExternalInput", name="woT", uniquify=False)
            wk3_d = dram.tile([CK, 9], f32, kind="ExternalInput", name="wk3T", uniquify=False)
            wdw_d = dram.tile([CK, 9], f32, kind="ExternalInput", name="wdw9", uniquify=False)
            bk1_d = dram.tile([CK, 1], f32, kind="ExternalInput", name="bk1c", uniquify=False)
            bv_d = dram.tile([CK, 1], f32, kind="ExternalInput", name="bvc", uniquify=False)
            bdw_d = dram.tile([CK, 1], f32, kind="ExternalInput", name="bdwc", uniquify=False)
            bk3_d = dram.tile([9, 1], f32, kind="ExternalInput", name="bk3c", uniquify=False)
            bo_d = dram.tile([128, 2], f32, kind="ExternalInput", name="boc", uniquify=False)
            ones9_d = dram.tile([9, 1], f32, kind="ExternalInput", name="ones9", uniquify=False)
            ones19_d = dram.tile([1, 9], f32, kind="ExternalInput", name="ones19", uniquify=False)
            bcast_d = dram.tile([9, 288], f32, kind="ExternalInput", name="bcast", uniquify=False)

            with (
                tc.tile_pool(name="consts", bufs=1) as cpool,
                tc.tile_pool(name="xp", bufs=3) as xpool,
                tc.tile_pool(name="kvp", bufs=3) as kvpool,
                tc.tile_pool(name="scr", bufs=3) as scpool,
                tc.tile_pool(name="outp", bufs=3) as outpool,
                tc.tile_pool(name="ps_conv", bufs=2, space="PSUM") as psA,
                tc.tile_pool(name="ps_small", bufs=2, space="PSUM") as psS,
                tc.tile_pool(name="ps_wb", bufs=2, space="PSUM") as psW,
                tc.tile_pool(name="ps_out", bufs=2, space="PSUM") as psO,
            ):
                # ---- constants into SBUF ----
                wk1s = cpool.tile([128, 2, CK], f32, name="wk1s")
                wvs = cpool.tile([128, 2, CK], f32, name="wvs")
                for h in range(2):
                    nc.sync.dma_start(out=wk1s[:, h, :], in_=wk1_d[h])
                    nc.sync.dma_start(out=wvs[:, h, :], in_=wv_d[h])
                wos = cpool.tile([CK, C], f32, name="wos")
                nc.sync.dma_start(out=wos[:], in_=wo_d[:])
                wk3s = cpool.tile([CK, 9], f32, name="wk3s")
                nc.sync.dma_start(out=wk3s[:], in_=wk3_d[:])
                wdws = cpool.tile([CK, 9], f32, name="wdws")
                nc.sync.dma_start(out=wdws[:], in_=wdw_d[:])
                bk1s = cpool.tile([CK, 1], f32, name="bk1s")
                nc.sync.dma_start(out=bk1s[:], in_=bk1_d[:])
                bvs = cpool.tile([CK, 1], f32, name="bvs")
                nc.sync.dma_start(out=bvs[:], in_=bv_d[:])
                bdws = cpool.tile([CK, 1], f32, name="bdws")
                nc.sync.dma_start(out=bdws[:], in_=bdw_d[:])
                bk3s = cpool.tile([9, 1], f32, name="bk3s")
                nc.sync.dma_start(out=bk3s[:], in_=bk3_d[:])
                bos = cpool.tile([128, 2], f32, name="bos")
                nc.sync.dma_start(out=bos[:], in_=bo_d[:])
                ones9s = cpool.tile([9, 1], f32, name="ones9s")
                nc.sync.dma_start(out=ones9s[:], in_=ones9_d[:])
                ones19s = cpool.tile([1, 9], f32, name="ones19s")
                nc.sync.dma_start(out=ones19s[:], in_=ones19_d[:])
                bcasts = cpool.tile([9, 288], f32, name="bcasts")
                nc.sync.dma_start(out=bcasts[:], in_=bcast_d[:])

                xt = [None] * T    # (x_lo, x_hi) per tile
                kvt = [None] * T   # (k1, v) per tile

                def emit_A(t):
                    x_lo = xpool.tile([128, R, W], f32, name="x_lo")
                    x_hi = xpool.tile([128, R, W], f32, name="x_hi")
                    nc.sync.dma_start(out=x_lo[:], in_=x_d[0:128, t * R:(t + 1) * R, :])
                    nc.sync.dma_start(out=x_hi[:], in_=x_d[128:256, t * R:(t + 1) * R, :])
                    k1 = kvpool.tile([CK, R + 2, WP], f32, name="k1")
                    v = kvpool.tile([CK, R + 2, WP], f32, name="v")
                    # zero the left/right pad columns
                    nc.gpsimd.memset(k1[:, :, 0:1], 0.0)
                    nc.gpsimd.memset(k1[:, :, WP - 1:WP], 0.0)
                    nc.gpsimd.memset(v[:, :, 0:1], 0.0)
                    nc.gpsimd.memset(v[:, :, WP - 1:WP], 0.0)
                    for q in range(NCH):
                        xl = x_lo[:, q * CR:(q + 1) * CR, :]
                        xh = x_hi[:, q * CR:(q + 1) * CR, :]
                        kp = psA.tile([CK, CR, W], f32, name="kp", tag="ps_conv")
                        nc.tensor.matmul(kp[:], mm(wk1s[:, 0, :]), mm(xl), start=True, stop=False)
                        nc.tensor.matmul(kp[:], mm(wk1s[:, 1, :]), mm(xh), start=False, stop=True)
                        nc.scalar.activation(k1[:, 1 + q * CR:1 + (q + 1) * CR, 1:1 + W],
                                             kp[:], Act.Relu, bias=bk1s[:, 0:1])
                        vp = psA.tile([CK, CR, W], f32, name="vp", tag="ps_conv")
                        nc.tensor.matmul(vp[:], mm(wvs[:, 0, :]), mm(xl), start=True, stop=False)
                        nc.tensor.matmul(vp[:], mm(wvs[:, 1, :]), mm(xh), start=False, stop=True)
                        nc.scalar.activation(v[:, 1 + q * CR:1 + (q + 1) * CR, 1:1 + W],
                                             vp[:], Act.Identity, bias=bvs[:, 0:1])
                    xt[t] = (x_lo, x_hi)
                    kvt[t] = (k1, v)

                def emit_B(u):
                    k1, v = kvt[u]
                    # fill halo rows (row 0 = image row u*R-1, row R+1 = image row u*R+R)
                    if u > 0:
                        pk1, pv = kvt[u - 1]
                        nc.scalar.copy(k1[:, 0, :], pk1[:, R, :])
                        nc.scalar.copy(v[:, 0, :], pv[:, R, :])
                    else:
                        nc.gpsimd.memset(k1[:, 0, :], 0.0)
                        nc.gpsimd.memset(v[:, 0, :], 0.0)
                    if u < T - 1:
                        nk1, nv = kvt[u + 1]
                        nc.scalar.copy(k1[:, R + 1, :], nk1[:, 1, :])
                        nc.scalar.copy(v[:, R + 1, :], nv[:, 1, :])
                    else:
                        nc.gpsimd.memset(k1[:, R + 1, :], 0.0)
                        nc.gpsimd.memset(v[:, R + 1, :], 0.0)

                    x_lo, x_hi = xt[u]
                    out_lo = outpool.tile([128, R, W], f32, name="out_lo")
                    out_hi = outpool.tile([128, R, W], f32, name="out_hi")

                    for q in range(NCH):
                        r0 = 1 + q * CR  # local row of first output row of chunk

                        # depthwise 3x3 on k1 -> k2
                        k2 = scpool.tile([CK, CR, W], f32, name="k2")
                        for jj, (dy, dx) in enumerate(TAPS):
                            kv_view = k1[:, r0 + dy:r0 + dy + CR, 1 + dx:1 + dx + W]
                            if jj == 0:
                                nc.vector.tensor_scalar(k2[:], kv_view,
                                                        wdws[:, 0:1], bdws[:, 0:1],
                                                        Alu.mult, Alu.add)
                            else:
                                nc.vector.scalar_tensor_tensor(k2[:], kv_view,
                                                               wdws[:, jj:jj + 1], k2[:],
                                                               Alu.mult, Alu.add)

                        # logits -> exp -> sum -> reciprocal -> normalized w9
                        lg = psS.tile([9, CR, W], f32, name="lg", tag="ps_small")
                        nc.tensor.matmul(lg[:], mm(wk3s[:]), mm(k2[:]), start=True, stop=True)
                        e = scpool.tile([9, CR, W], f32, name="e")
                        nc.scalar.activation(e[:], lg[:], Act.Exp, bias=bk3s[:, 0:1])
                        S = psS.tile([1, CR, W], f32, name="S", tag="ps_small")
                        nc.tensor.matmul(S[:], mm(ones9s[:]), mm(e[:]), start=True, stop=True)
                        rc = scpool.tile([1, CR, W], f32, name="rc")
                        nc.vector.reciprocal(rc[:], S[:])
                        r9 = psS.tile([9, CR, W], f32, name="r9", tag="ps_small")
                        nc.tensor.matmul(r9[:], mm(ones19s[:]), mm(rc[:]), start=True, stop=True)
                        w9 = scpool.tile([9, CR, W], f32, name="w9")
                        nc.vector.tensor_tensor(w9[:], e[:], r9[:], Alu.mult)

                        # aggregation: y[c,p] = sum_j w9[j,p] * v[c, p+off_j]
                        y_acc = scpool.tile([CK, CR, W], f32, name="y_acc")
                        for g in range(3):
                            wb = psW.tile([96, CR, W], f32, name="wb", tag="ps_wb")
                            nc.tensor.matmul(wb[:], mm(bcasts[:, 96 * g:96 * (g + 1)]),
                                             mm(w9[:]), start=True, stop=True)
                            for a in range(3):
                                jj = 3 * g + a
                                dy, dx = TAPS[jj]
                                v_view = v[:, r0 + dy:r0 + dy + CR, 1 + dx:1 + dx + W]
                                wbs = wb[32 * a:32 * (a + 1), :, :]
                                if jj == 0:
                                    nc.vector.tensor_tensor(y_acc[:], wbs, v_view, Alu.mult)
                                else:
                                    pr = scpool.tile([CK, CR, W], f32, name="pr")
                                    nc.vector.tensor_tensor(pr[:], wbs, v_view, Alu.mult)
                                    nc.gpsimd.tensor_tensor(y_acc[:], y_acc[:], pr[:], Alu.add)

                        # out conv + bias + residual
                        op_lo = psO.tile([128, CR, W], f32, name="op_lo", tag="ps_out")
                        nc.tensor.matmul(op_lo[:], mm(wos[:, 0:128]), mm(y_acc[:]),
                                         start=True, stop=True)
                        nc.vector.scalar_tensor_tensor(out_lo[:, q * CR:(q + 1) * CR, :],
                                                       op_lo[:], bos[:, 0:1],
                                                       x_lo[:, q * CR:(q + 1) * CR, :],
                                                       Alu.add, Alu.add)
                        op_hi = psO.tile([128, CR, W], f32, name="op_hi", tag="ps_out")
                        nc.tensor.matmul(op_hi[:], mm(wos[:, 128:256]), mm(y_acc[:]),
                                         start=True, stop=True)
                        nc.vector.scalar_tensor_tensor(out_hi[:, q * CR:(q + 1) * CR, :],
                                                       op_hi[:], bos[:, 1:2],
                                                       x_hi[:, q * CR:(q + 1) * CR, :],
                                                       Alu.add, Alu.add)

                    nc.sync.dma_start(out=out_d[0:128, u * R:(u + 1) * R, :], in_=out_lo[:])
                    nc.sync.dma_start(out=out_d[128:256, u * R:(u + 1) * R, :], in_=out_hi[:])

                emit_A(0)
                emit_A(1)
                for t in range(2, T):
                    emit_A(t)
                    emit_B(t - 2)
                emit_B(T - 2)
                emit_B(T - 1)

    nc.compile()
    return nc


def build_nc_v2(mm_dtype=MM_DTYPE, reps=1):
    """Partition-packed variant: 4 row-groups x 32 channels = 128 partitions.

    Each 16-row tile is processed as 4 groups of 4 rows; group a's
    channel-c data lives on partition 32a+c.  Matmuls use tile_position
    col/row groups so the 4 per-group matmuls pack into the PE array and
    one PSUM bank; elementwise ops run on all 128 partitions (4x fewer
    DVE/Pool ops than the unpacked variant).  k1p/vp tiles hold 6 local
    rows per group (1-row halo duplicated between neighbouring groups by
    SBUF->SBUF DMA, cross-tile for group 0/3 edges).
    """
    from concourse import bacc
    import concourse.mybir as mybir
    import concourse.tile as tile

    dt = mybir.dt
    f32 = dt.float32
    bf16 = dt.bfloat16
    Alu = mybir.AluOpType
    Act = mybir.ActivationFunctionType

    nc = bacc.Bacc(None, target_bir_lowering=False, debug=True)

    with tile.TileContext(nc) as tc:
        with tc.tile_pool(name="dram", bufs=1, space="DRAM") as dram:
            x_d = dram.tile([C, H, W], f32, kind="ExternalInput", name="x", uniquify=False)
            out_d = dram.tile([C, H, W], f32, kind="ExternalOutput", name="out", uniquify=False)
            wk1_d = dram.tile([2, 128, CK], f32, kind="ExternalInput", name="wk1T", uniquify=False)
            wv_d = dram.tile([2, 128, CK], f32, kind="ExternalInput", name="wvT", uniquify=False)
            wop_d = dram.tile([128, C], bf16, kind="ExternalInput", name="wop", uniquify=False)
            wk3p_d = dram.tile([128, 9], bf16, kind="ExternalInput", name="wk3p", uniquify=False)
            wdwp_d = dram.tile([128, 9], f32, kind="ExternalInput", name="wdwp", uniquify=False)
            bk1p_d = dram.tile([128, 1], f32, kind="ExternalInput", name="bk1p", uniquify=False)
            bvp_d = dram.tile([128, 1], f32, kind="ExternalInput", name="bvp", uniquify=False)
            bdwp_d = dram.tile([128, 1], f32, kind="ExternalInput", name="bdwp", uniquify=False)
            bk3p_d = dram.tile([128, 1], f32, kind="ExternalInput", name="bk3p", uniquify=False)
            bo_d = dram.tile([128, 2], f32, kind="ExternalInput", name="boc", uniquify=False)
            ones_d = dram.tile([128, 32], bf16, kind="ExternalInput", name="ones32", uniquify=False)
            bc2_d = dram.tile([128, 288], bf16, kind="ExternalInput", name="bc2", uniquify=False)

            with (
                tc.tile_pool(name="consts", bufs=1) as cpool,
                tc.tile_pool(name="xp", bufs=3) as xpool,
                tc.tile_pool(name="kvp", bufs=3) as kvpool,
                tc.tile_pool(name="scr", bufs=3) as scpool,
                tc.tile_pool(name="outp", bufs=3) as outpool,
                tc.tile_pool(name="ps_conv", bufs=2, space="PSUM") as psA,
                tc.tile_pool(name="ps_small", bufs=2, space="PSUM") as psS,
                tc.tile_pool(name="ps_wb", bufs=2, space="PSUM") as psW,
                tc.tile_pool(name="ps_out", bufs=2, space="PSUM") as psO,
            ):
                wk1s = cpool.tile([128, 2, CK], f32, name="wk1s")
                wvs = cpool.tile([128, 2, CK], f32, name="wvs")
                for h in range(2):
                    nc.sync.dma_start(out=wk1s[:, h, :], in_=wk1_d[h])
                    nc.sync.dma_start(out=wvs[:, h, :], in_=wv_d[h])
                wops = cpool.tile([128, C], bf16, name="wops")
                nc.sync.dma_start(out=wops[:], in_=wop_d[:])
                wk3s = cpool.tile([128, 9], bf16, name="wk3s")
                nc.sync.dma_start(out=wk3s[:], in_=wk3p_d[:])
                wdws = cpool.tile([128, 9], f32, name="wdws")
                nc.sync.dma_start(out=wdws[:], in_=wdwp_d[:])
                bk1s = cpool.tile([128, 1], f32, name="bk1s")
                nc.sync.dma_start(out=bk1s[:], in_=bk1p_d[:])
                bvs = cpool.tile([128, 1], f32, name="bvs")
                nc.sync.dma_start(out=bvs[:], in_=bvp_d[:])
                bdws = cpool.tile([128, 1], f32, name="bdws")
                nc.sync.dma_start(out=bdws[:], in_=bdwp_d[:])
                bk3s = cpool.tile([128, 1], f32, name="bk3s")
                nc.sync.dma_start(out=bk3s[:], in_=bk3p_d[:])
                bos = cpool.tile([128, 2], f32, name="bos")
                nc.sync.dma_start(out=bos[:], in_=bo_d[:])
                oness = cpool.tile([128, 32], bf16, name="oness")
                nc.sync.dma_start(out=oness[:], in_=ones_d[:])
                bc2s = cpool.tile([128, 288], bf16, name="bc2s")
                nc.sync.dma_start(out=bc2s[:], in_=bc2_d[:])

                xt = [None] * T
                kvt = [None] * T

                def emit_A(t):
                    x_lo = xpool.tile([128, R, W], f32, name="x_lo")
                    x_hi = xpool.tile([128, R, W], f32, name="x_hi")
                    nc.sync.dma_start(out=x_lo[:], in_=x_d[0:128, t * R:(t + 1) * R, :])
                    nc.sync.dma_start(out=x_hi[:], in_=x_d[128:256, t * R:(t + 1) * R, :])
                    kv = kvpool.tile([128, 2, 6, WP], bf16, name="kv")
                    k1p = kv[:, 0]
                    vp = kv[:, 1]
                    nc.gpsimd.memset(kv[:, :, :, 0:1], 0.0)
                    nc.gpsimd.memset(kv[:, :, :, WP - 1:WP], 0.0)
                    kp = psA.tile([128, CR, W], f32, name="kp", tag="ps_conv")
                    vpp = psA.tile([128, CR, W], f32, name="vpp", tag="ps_conv")
                    for a in range(4):
                        xl = x_lo[:, 4 * a:4 * a + 4, :]
                        xh = x_hi[:, 4 * a:4 * a + 4, :]
                        po = kp[32 * a:32 * (a + 1), :, :]
                        nc.tensor.matmul(po, wk1s[:, 0, :], xl,
                                         start=True, stop=False, tile_position=(0, 32 * a))
                        nc.tensor.matmul(po, wk1s[:, 1, :], xh,
                                         start=False, stop=True, tile_position=(0, 32 * a))
                        po = vpp[32 * a:32 * (a + 1), :, :]
                        nc.tensor.matmul(po, wvs[:, 0, :], xl,
                                         start=True, stop=False, tile_position=(0, 32 * a))
                        nc.tensor.matmul(po, wvs[:, 1, :], xh,
                                         start=False, stop=True, tile_position=(0, 32 * a))
                    nc.scalar.activation(k1p[:, 1:5, 1:1 + W], kp[:], Act.Relu,
                                         bias=bk1s[:, 0:1])
                    nc.scalar.activation(vp[:, 1:5, 1:1 + W], vpp[:], Act.Identity,
                                         bias=bvs[:, 0:1])
                    # duplicate halo rows between neighbouring groups (intra-tile)
                    nc.sync.dma_start(out=kv[32:128, :, 0, :], in_=kv[0:96, :, 4, :])
                    nc.sync.dma_start(out=kv[0:96, :, 5, :], in_=kv[32:128, :, 1, :])
                    xt[t] = (x_lo, x_hi)
                    kvt[t] = kv

                def emit_B(u):
                    kv = kvt[u]
                    k1p = kv[:, 0]
                    vp = kv[:, 1]
                    # cross-tile halo rows for group 0 (top) and group 3 (bottom)
                    if u > 0:
                        nc.sync.dma_start(out=kv[0:32, :, 0, :],
                                          in_=kvt[u - 1][96:128, :, 4, :])
                    else:
                        nc.gpsimd.memset(kv[0:32, :, 0, :], 0.0)
                    if u < T - 1:
                        nc.sync.dma_start(out=kv[96:128, :, 5, :],
                                          in_=kvt[u + 1][0:32, :, 1, :])
                    else:
                        nc.gpsimd.memset(kv[96:128, :, 5, :], 0.0)

                    x_lo, x_hi = xt[u]
                    out_lo = outpool.tile([128, R, W], f32, name="out_lo")
                    out_hi = outpool.tile([128, R, W], f32, name="out_hi")

                    def kview(tt, jj):
                        dy, dx = TAPS[jj]
                        return tt[:, 1 + dy:5 + dy, 1 + dx:1 + dx + W]

                    # depthwise 3x3: 9 taps chained on DVE (bf16, 2x mode)
                    k2m = scpool.tile([128, CR, W], bf16, name="k2m")
                    with nc.allow_low_precision(reason="bf16 dwconv accumulation"):
                        nc.vector.tensor_scalar(k2m[:], kview(k1p, 0), wdws[:, 0:1],
                                                bdws[:, 0:1], Alu.mult, Alu.add)
                        for jj in range(1, 9):
                            nc.vector.scalar_tensor_tensor(k2m[:], kview(k1p, jj),
                                                           wdws[:, jj:jj + 1], k2m[:],
                                                           Alu.mult, Alu.add)

                    # logits / exp / sum / recip per group (tile-packed matmuls)
                    lg = psS.tile([128, CR, W], f32, name="lg", tag="ps_small")
                    for a in range(4):
                        nc.tensor.matmul(lg[32 * a:32 * a + 9, :, :],
                                         wk3s[32 * a:32 * (a + 1), :],
                                         k2m[32 * a:32 * (a + 1), :, :],
                                         start=True, stop=True,
                                         tile_position=(32 * a, 32 * a))
                    e = scpool.tile([128, CR, W], bf16, name="e")
                    for a in range(4):
                        nc.scalar.activation(e[32 * a:32 * a + 9, :, :],
                                             lg[32 * a:32 * a + 9, :, :], Act.Exp,
                                             bias=bk3s[32 * a:32 * a + 9, 0:1])
                    S = psS.tile([128, CR, W], f32, name="S", tag="ps_small")
                    for a in range(4):
                        nc.tensor.matmul(S[32 * a:32 * a + 1, :, :],
                                         oness[32 * a:32 * a + 9, 0:1],
                                         e[32 * a:32 * a + 9, :, :],
                                         start=True, stop=True,
                                         tile_position=(32 * a, 32 * a))
                    rc = scpool.tile([128, CR, W], bf16, name="rc")
                    with nc.allow_low_precision(reason="bf16 softmax recip"):
                        for a in range(4):
                            nc.vector.reciprocal(rc[32 * a:32 * a + 1, :, :],
                                                 S[32 * a:32 * a + 1, :, :])
                    rb = psS.tile([128, CR, W], f32, name="rb", tag="ps_small")
                    for a in range(4):
                        nc.tensor.matmul(rb[32 * a:32 * (a + 1), :, :],
                                         oness[32 * a:32 * a + 1, 0:32],
                                         rc[32 * a:32 * a + 1, :, :],
                                         start=True, stop=True,
                                         tile_position=(32 * a, 32 * a))
                    # aggregation with unnormalized weights, normalize at the end;
                    # products on DVE, pairwise tree-adds on Pool
                    prods = []
                    for jj in range(9):
                        wb = psW.tile([128, CR, W], f32, name="wb", tag="ps_wb")
                        for a in range(4):
                            nc.tensor.matmul(wb[32 * a:32 * (a + 1), :, :],
                                             bc2s[32 * a:32 * a + 9, 32 * jj:32 * (jj + 1)],
                                             e[32 * a:32 * a + 9, :, :],
                                             start=True, stop=True,
                                             tile_position=(32 * a, 32 * a))
                        vv = kview(vp, jj)
                        pr = scpool.tile([128, CR, W], f32, name="pr", bufs=6)
                        nc.vector.tensor_tensor(pr[:], wb[:], vv, Alu.mult)
                        prods.append(pr)
                        if jj % 2 == 1:   # fold pairs as they arrive
                            nc.gpsimd.tensor_tensor(prods[jj - 1][:], prods[jj - 1][:],
                                                    prods[jj][:], Alu.add)
                    s0, s1, s2, s3, p8 = prods[0], prods[2], prods[4], prods[6], prods[8]
                    nc.gpsimd.tensor_tensor(s3[:], s3[:], p8[:], Alu.add)
                    nc.gpsimd.tensor_tensor(s0[:], s0[:], s1[:], Alu.add)
                    nc.gpsimd.tensor_tensor(s2[:], s2[:], s3[:], Alu.add)
                    nc.gpsimd.tensor_tensor(s0[:], s0[:], s2[:], Alu.add)
                    y_bf = scpool.tile([128, CR, W], bf16, name="y_bf")
                    nc.vector.tensor_tensor(y_bf[:], s0[:], rb[:], Alu.mult)

                    # out conv + bias via ACT, residual add on Pool
                    for half, (xh, outh) in enumerate(((x_lo, out_lo), (x_hi, out_hi))):
                        for a in range(4):
                            op = psO.tile([128, CR, W], f32, name="op", tag="ps_out")
                            nc.tensor.matmul(op[:],
                                             wops[32 * a:32 * (a + 1), 128 * half:128 * (half + 1)],
                                             y_bf[32 * a:32 * (a + 1), :, :],
                                             start=True, stop=True,
                                             tile_position=(32 * a, 0))
                            ov = outh[:, 4 * a:4 * a + 4, :]
                            nc.scalar.activation(ov, op[:], Act.Identity,
                                                 bias=bos[:, half:half + 1])
                            nc.gpsimd.tensor_tensor(ov, ov, xh[:, 4 * a:4 * a + 4, :],
                                                    Alu.add)

                    nc.sync.dma_start(out=out_d[0:128, u * R:(u + 1) * R, :], in_=out_lo[:])
                    nc.sync.dma_start(out=out_d[128:256, u * R:(u + 1) * R, :], in_=out_hi[:])

                def emit_all():
                    for i in range(T):
                        xt[i] = None
                        kvt[i] = None
                    emit_A(0)
                    emit_A(1)
                    for t in range(2, T):
                        emit_A(t)
                        emit_B(t - 2)
                    emit_B(T - 2)
                    emit_B(T - 1)

                if reps > 1:
                    with tc.For_i(0, reps, 1):
                        emit_all()
                else:
                    emit_all()

    nc.compile()
    return nc


def build_nc_v3():
    """v3: blockdiag weights instead of tile_position packing, f32r input
    convs, dwconv+Wk3 fused into 9 accumulating matmuls, packed exp,
    reciprocal_approx_fast, residual via identity matmul into PSUM.

    Layout identical to v2: 4 row-groups x 32 channels on 128 partitions,
    T=8 tiles of R=16 rows, kv tiles [128, 2, 6, 130] bf16 with halo rows.
    Group-a quantities that are per-pixel scalars live at partitions 9a+o
    (logits/e, o<9) or a (sums), via block-diagonal lhsT weights.
    """
    from concourse import bacc
    import concourse.mybir as mybir
    import concourse.tile as tile

    dt = mybir.dt
    f32 = dt.float32
    f32r = dt.float32r
    bf16 = dt.bfloat16
    f8 = dt.float8e4
    DR = mybir.MatmulPerfMode.DoubleRow
    Alu = mybir.AluOpType
    Act = mybir.ActivationFunctionType

    def r(ap):
        return ap.bitcast(f32r)

    nc = bacc.Bacc(None, target_bir_lowering=False, debug=True)

    with tile.TileContext(nc) as tc:
        with tc.tile_pool(name="dram", bufs=1, space="DRAM") as dram:
            x_d = dram.tile([C, H, W], f32, kind="ExternalInput", name="x", uniquify=False)
            out_d = dram.tile([C, H, W], f32, kind="ExternalOutput", name="out", uniquify=False)
            wk1_d = dram.tile([2, 128, CK], bf16, kind="ExternalInput", name="wk1T", uniquify=False)
            wv_d = dram.tile([2, 128, CK], bf16, kind="ExternalInput", name="wvT", uniquify=False)
            wop_d = dram.tile([128, C], bf16, kind="ExternalInput", name="wop", uniquify=False)
            mf_d = dram.tile([128, 9, 36], bf16, kind="ExternalInput", name="mf", uniquify=False)
            wbm_d = dram.tile([36, 9, 128], bf16, kind="ExternalInput", name="wbm", uniquify=False)
            sones_d = dram.tile([36, 4], bf16, kind="ExternalInput", name="sones", uniquify=False)
            bk1p_d = dram.tile([128, 1], f32, kind="ExternalInput", name="bk1p", uniquify=False)
            bvp_d = dram.tile([128, 1], f32, kind="ExternalInput", name="bvp", uniquify=False)
            bk3f_d = dram.tile([36, 1], f32, kind="ExternalInput", name="bk3f", uniquify=False)
            bo_d = dram.tile([128, 2], f32, kind="ExternalInput", name="boc", uniquify=False)

            with (
                tc.tile_pool(name="consts", bufs=1) as cpool,
                tc.tile_pool(name="xbp", bufs=5) as xbpool,
                tc.tile_pool(name="kvp", bufs=4) as kvpool,
                tc.tile_pool(name="scr", bufs=3) as scpool,
                tc.tile_pool(name="outp", bufs=3) as outpool,
                tc.tile_pool(name="ps_conv", bufs=2, space="PSUM") as psA,
                tc.tile_pool(name="ps_small", bufs=2, space="PSUM") as psS,
                tc.tile_pool(name="ps_wb", bufs=2, space="PSUM") as psW,
                tc.tile_pool(name="ps_out", bufs=2, space="PSUM") as psO,

            ):
                wk1s = cpool.tile([128, 2, CK], bf16, name="wk1s")
                wvs = cpool.tile([128, 2, CK], bf16, name="wvs")
                for h in range(2):
                    nc.sync.dma_start(out=wk1s[:, h, :], in_=wk1_d[h])
                    nc.sync.dma_start(out=wvs[:, h, :], in_=wv_d[h])
                wops = cpool.tile([128, C], bf16, name="wops")
                nc.sync.dma_start(out=wops[:], in_=wop_d[:])
                mfs = cpool.tile([128, 9, 36], bf16, name="mfs")
                nc.sync.dma_start(out=mfs[:], in_=mf_d[:])
                wbms = cpool.tile([36, 9, 128], bf16, name="wbms")
                nc.sync.dma_start(out=wbms[:], in_=wbm_d[:])
                soness = cpool.tile([36, 4], bf16, name="soness")
                nc.sync.dma_start(out=soness[:], in_=sones_d[:])
                bk1s = cpool.tile([128, 1], f32, name="bk1s")
                nc.sync.dma_start(out=bk1s[:], in_=bk1p_d[:])
                bvs = cpool.tile([128, 1], f32, name="bvs")
                nc.sync.dma_start(out=bvs[:], in_=bvp_d[:])
                bk3s = cpool.tile([36, 1], f32, name="bk3s")
                nc.sync.dma_start(out=bk3s[:], in_=bk3f_d[:])
                bos = cpool.tile([128, 2], f32, name="bos")
                nc.sync.dma_start(out=bos[:], in_=bo_d[:])

                xt = [None] * T
                kvt = [None] * T
                et = [None] * T

                def emit_A(t):
                    xb_lo = xbpool.tile([128, R, W], bf16, name="xb_lo")
                    xb_hi = xbpool.tile([128, R, W], bf16, name="xb_hi")
                    nc.gpsimd.dma_start(out=xb_lo[:], in_=x_d[0:128, t * R:(t + 1) * R, :])
                    nc.gpsimd.dma_start(out=xb_hi[:], in_=x_d[128:256, t * R:(t + 1) * R, :])
                    kv = kvpool.tile([128, 2, 6, WP], bf16, name="kv")
                    k1p = kv[:, 0]
                    vp = kv[:, 1]
                    nc.gpsimd.memset(kv[:, :, :, 0:1], 0.0)
                    nc.gpsimd.memset(kv[:, :, :, WP - 1:WP], 0.0)
                    kp = psA.tile([128, CR, W], f32, name="kp", tag="ps_conv")
                    vpp = psA.tile([128, CR, W], f32, name="vpp", tag="ps_conv")
                    for a in range(4):
                        xl = xb_lo[:, 4 * a:4 * a + 4, :]
                        xh = xb_hi[:, 4 * a:4 * a + 4, :]
                        po = kp[32 * a:32 * (a + 1), :, :]
                        nc.tensor.matmul(po, wk1s[:, 0, :], xl,
                                         start=True, stop=False, tile_position=(0, 32 * a))
                        nc.tensor.matmul(po, wk1s[:, 1, :], xh,
                                         start=False, stop=True, tile_position=(0, 32 * a))
                        po = vpp[32 * a:32 * (a + 1), :, :]
                        nc.tensor.matmul(po, wvs[:, 0, :], xl,
                                         start=True, stop=False, tile_position=(0, 32 * a))
                        nc.tensor.matmul(po, wvs[:, 1, :], xh,
                                         start=False, stop=True, tile_position=(0, 32 * a))
                    nc.scalar.activation(k1p[:, 1:5, 1:1 + W], kp[:], Act.Relu,
                                         bias=bk1s[:, 0:1])
                    nc.scalar.activation(vp[:, 1:5, 1:1 + W], vpp[:], Act.Identity,
                                         bias=bvs[:, 0:1])
                    nc.scalar.dma_start(out=kv[32:128, :, 0, :], in_=kv[0:96, :, 4, :])
                    nc.scalar.dma_start(out=kv[0:96, :, 5, :], in_=kv[32:128, :, 1, :])
                    xt[t] = (xb_lo, xb_hi)
                    kvt[t] = kv

                def emit_B1(u):
                    """Softmax pipe: halos, fused dwconv+Wk3 logits, exp,
                    sum, reciprocal, reciprocal broadcast."""
                    kv = kvt[u]
                    k1p = kv[:, 0]
                    if u > 0:
                        nc.scalar.dma_start(out=kv[0:32, :, 0, :],
                                            in_=kvt[u - 1][96:128, :, 4, :])
                    else:
                        nc.gpsimd.memset(kv[0:32, :, 0, :], 0.0)
                    if u < T - 1:
                        nc.scalar.dma_start(out=kv[96:128, :, 5, :],
                                            in_=kvt[u + 1][0:32, :, 1, :])
                    else:
                        nc.gpsimd.memset(kv[96:128, :, 5, :], 0.0)

                    def kview(tt, jj):
                        dy, dx = TAPS[jj]
                        return tt[:, 1 + dy:5 + dy, 1 + dx:1 + dx + W]

                    lg = psS.tile([36, CR, W], f32, name="lg", tag="ps_small")
                    for jj in range(9):
                        nc.tensor.matmul(lg[:], mfs[:, jj, :], kview(k1p, jj),
                                         start=(jj == 0), stop=(jj == 8))
                    e = scpool.tile([36, CR, W], bf16, name="e")
                    with nc.allow_low_precision(reason="bf16 softmax weights"):
                        nc.scalar.activation(e[:], lg[:], Act.Exp, bias=bk3s[:, 0:1])
                    S = psS.tile([4, CR, W], f32, name="S", tag="ps_small")
                    nc.tensor.matmul(S[:], soness[:], e[:], start=True, stop=True)
                    rc = scpool.tile([4, CR, W], f32, name="rc")
                    with nc.allow_low_precision(reason="approx reciprocal"):
                        nc.vector.reciprocal_approx_fast(out=rc[:], in_=S[:])
                    rc_bf = scpool.tile([4, CR, W], bf16, name="rc_bf")
                    with nc.allow_low_precision(reason="bf16 softmax recip"):
                        nc.scalar.copy(rc_bf[:], rc[:])
                    rb_sb = scpool.tile([128, CR, W], bf16, name="rb_sb")
                    nc.scalar.dma_start(
                        out=rb_sb[:],
                        in_=rc_bf[:].unsqueeze(1).broadcast_to((4, 32, CR, W)))
                    et[u] = (e, rb_sb)

                def emit_B2(u):
                    """Aggregation + out conv + residual + store."""
                    kv = kvt[u]
                    vp = kv[:, 1]
                    e, rb = et[u]
                    x_lo, x_hi = xt[u]
                    out_lo = outpool.tile([128, R, W], f32, name="out_lo")
                    out_hi = outpool.tile([128, R, W], f32, name="out_hi")

                    def kview(tt, jj):
                        dy, dx = TAPS[jj]
                        return tt[:, 1 + dy:5 + dy, 1 + dx:1 + dx + W]

                    prods = []
                    for jj in range(9):
                        wb = psW.tile([128, CR, W], f32, name="wb", tag="ps_wb")
                        nc.tensor.matmul(wb[:], wbms[:, jj, :], e[:],
                                         start=True, stop=True)
                        vv = kview(vp, jj)
                        pr = scpool.tile([128, CR, W], bf16, name="pr", bufs=6)
                        with nc.allow_low_precision(reason="bf16 aggregation"):
                            nc.vector.tensor_tensor(pr[:], wb[:], vv, Alu.mult)
                        prods.append(pr)
                        if jj % 2 == 1:
                            nc.vector.tensor_tensor(prods[jj - 1][:], prods[jj - 1][:],
                                                    prods[jj][:], Alu.add)
                    s0, s1, s2, s3, p8 = prods[0], prods[2], prods[4], prods[6], prods[8]
                    nc.gpsimd.tensor_tensor(s3[:], s3[:], p8[:], Alu.add)
                    nc.gpsimd.tensor_tensor(s0[:], s0[:], s1[:], Alu.add)
                    nc.gpsimd.tensor_tensor(s2[:], s2[:], s3[:], Alu.add)
                    nc.gpsimd.tensor_tensor(s0[:], s0[:], s2[:], Alu.add)
                    y_bf = scpool.tile([128, CR, W], bf16, name="y_bf")
                    with nc.allow_low_precision(reason="bf16 aggregation"):
                        nc.vector.tensor_tensor(y_bf[:], s0[:], rb[:], Alu.mult)

                    for half, (xh, outh) in enumerate(((x_lo, out_lo), (x_hi, out_hi))):
                        for a in range(4):
                            op = psO.tile([128, CR, W], f32, name="op", tag="ps_out")
                            nc.tensor.matmul(op[:],
                                             wops[32 * a:32 * (a + 1), 128 * half:128 * (half + 1)],
                                             y_bf[32 * a:32 * (a + 1), :, :],
                                             start=True, stop=True,
                                             tile_position=(32 * a, 0))
                            ov = outh[:, 4 * a:4 * a + 4, :]
                            if half == 0:
                                nc.vector.scalar_tensor_tensor(ov, op[:],
                                                               bos[:, 0:1],
                                                               xh[:, 4 * a:4 * a + 4, :],
                                                               Alu.add, Alu.add)
                            else:
                                nc.scalar.activation(ov, op[:], Act.Identity,
                                                     bias=bos[:, 1:2])
                                nc.gpsimd.tensor_tensor(ov, ov,
                                                        xh[:, 4 * a:4 * a + 4, :],
                                                        Alu.add)

                    nc.scalar.dma_start(out=out_d[0:128, u * R:(u + 1) * R, :], in_=out_lo[:])
                    nc.scalar.dma_start(out=out_d[128:256, u * R:(u + 1) * R, :], in_=out_hi[:])

                emit_A(0)
                emit_A(1)
                emit_B1(0)
                emit_A(2)
                for t in range(3, T):
                    emit_A(t)
                    emit_B2(t - 3)
                    emit_B1(t - 2)
                emit_B1(T - 2)
                emit_B2(T - 3)
                emit_B1(T - 1)
                emit_B2(T - 2)
                emit_B2(T - 1)

    nc.compile()
    return nc


def make_const_inputs_v3(Wk1, bk1, Wdw, bdw, Wk3, bk3, Wv, bv, Wo, bo):
    import ml_dtypes
    f = np.float32
    bf = ml_dtypes.bfloat16
    f8 = ml_dtypes.float8_e4m3
    Wdw9 = Wdw.reshape(CK, 9).astype(f)       # [c, j]
    mf = np.zeros((128, 9, 36), f)
    for a in range(4):
        for c in range(CK):
            for j in range(9):
                # lg[9a+o] += sum_c Wk3[o,c]*Wdw[c,j] * k1[32a+c, p+delta_j]
                mf[32 * a + c, j, 9 * a:9 * a + 9] = Wk3[:, c] * Wdw9[c, j]
    bk3f = np.zeros((36, 1), f)
    for a in range(4):
        bk3f[9 * a:9 * a + 9, 0] = bk3 + Wk3 @ bdw
    sones = np.zeros((36, 4), f)
    for a in range(4):
        sones[9 * a:9 * a + 9, a] = 1.0
    wbm = np.zeros((36, 9, 128), f)
    for a in range(4):
        for j in range(9):
            wbm[9 * a + j, j, 32 * a:32 * (a + 1)] = 1.0
    return {
        "wk1T": np.ascontiguousarray(Wk1.T.reshape(2, 128, CK), f).astype(bf),
        "wvT": np.ascontiguousarray(Wv.T.reshape(2, 128, CK), f).astype(bf),
        "wop": np.ascontiguousarray(np.tile(Wo.T, (4, 1))).astype(bf),
        "mf": mf.astype(bf),
        "sones": sones.astype(bf),
        "wbm": wbm.astype(bf),
        "bk1p": np.ascontiguousarray(np.tile(bk1.reshape(CK, 1), (4, 1)), f),
        "bvp": np.ascontiguousarray(np.tile(bv.reshape(CK, 1), (4, 1)), f),
        "bk3f": bk3f,
        "boc": np.ascontiguousarray(bo.reshape(2, 128).T, f),
    }


def make_const_inputs_v2(Wk1, bk1, Wdw, bdw, Wk3, bk3, Wv, bv, Wo, bo):
    import ml_dtypes
    f = np.float32
    bf = ml_dtypes.bfloat16
    bc2 = np.zeros((128, 288), bf)
    for a in range(4):
        for j in range(9):
            bc2[32 * a + j, 32 * j:32 * (j + 1)] = 1.0
    bk3p = np.zeros((128, 1), f)
    for a in range(4):
        bk3p[32 * a:32 * a + 9, 0] = bk3
    return {
        "wk1T": np.ascontiguousarray(Wk1.T.reshape(2, 128, CK), f),
        "wvT": np.ascontiguousarray(Wv.T.reshape(2, 128, CK), f),
        "wop": np.ascontiguousarray(np.tile(Wo.T, (4, 1))).astype(bf),
        "wk3p": np.ascontiguousarray(np.tile(Wk3.T, (4, 1))).astype(bf),
        "wdwp": np.ascontiguousarray(np.tile(Wdw.reshape(CK, 9), (4, 1)), f),
        "bk1p": np.ascontiguousarray(np.tile(bk1.reshape(CK, 1), (4, 1)), f),
        "bvp": np.ascontiguousarray(np.tile(bv.reshape(CK, 1), (4, 1)), f),
        "bdwp": np.ascontiguousarray(np.tile(bdw.reshape(CK, 1), (4, 1)), f),
        "bk3p": bk3p,
        "boc": np.ascontiguousarray(bo.reshape(2, 128).T, f),
        "ones32": np.ones((128, 32), bf),
        "bc2": bc2,
    }


def make_const_inputs(Wk1, bk1, Wdw, bdw, Wk3, bk3, Wv, bv, Wo, bo):
    f = np.float32
    bcast = np.zeros((9, 288), f)
    for j in range(9):
        g, a = divmod(j, 3)
        bcast[j, 96 * g + 32 * a:96 * g + 32 * (a + 1)] = 1.0
    return {
        "wk1T": np.ascontiguousarray(Wk1.T.reshape(2, 128, CK), f),
        "wvT": np.ascontiguousarray(Wv.T.reshape(2, 128, CK), f),
        "woT": np.ascontiguousarray(Wo.T, f),
        "wk3T": np.ascontiguousarray(Wk3.T, f),
        "wdw9": np.ascontiguousarray(Wdw.reshape(CK, 9), f),
        "bk1c": np.ascontiguousarray(bk1.reshape(CK, 1), f),
        "bvc": np.ascontiguousarray(bv.reshape(CK, 1), f),
        "bdwc": np.ascontiguousarray(bdw.reshape(CK, 1), f),
        "bk3c": np.ascontiguousarray(bk3.reshape(9, 1), f),
        "boc": np.ascontiguousarray(bo.reshape(2, 128).T, f),
        "ones9": np.ones((9, 1), f),
        "ones19": np.ones((1, 9), f),
        "bcast": bcast,
    }


VERSION = 3

_NC_CACHE = {}


def build():
    if VERSION == 3:
        return build_nc_v3()
    return build_nc_v2(MM_DTYPE) if VERSION == 2 else build_nc(MM_DTYPE)


def consts(**kw):
    if VERSION == 3:
        return make_const_inputs_v3(**kw)
    fn = make_const_inputs_v2 if VERSION == 2 else make_const_inputs
    return fn(**kw)


def _get_nc():
    key = (VERSION, MM_DTYPE)
    if key not in _NC_CACHE:
        _NC_CACHE[key] = build()
    return _NC_CACHE[key]


LAST_RESULT = None


def kernel(x, Wk1, bk1, Wdw, bdw, Wk3, bk3, Wv, bv, Wo, bo):
    global LAST_RESULT
    from concourse.bass_utils import run_bass_kernel_spmd

    x = np.asarray(x, np.float32)
    B = x.shape[0]
    assert B == 8 and x.shape[1:] == (C, H, W)
    cs = consts(Wk1=np.asarray(Wk1), bk1=np.asarray(bk1), Wdw=np.asarray(Wdw),
                bdw=np.asarray(bdw), Wk3=np.asarray(Wk3), bk3=np.asarray(bk3),
                Wv=np.asarray(Wv), bv=np.asarray(bv), Wo=np.asarray(Wo),
                bo=np.asarray(bo))
    nc = _get_nc()
    in_maps = [dict(cs, x=np.ascontiguousarray(x[i])) for i in range(B)]
    res = run_bass_kernel_spmd(nc, in_maps, list(range(B)))
    LAST_RESULT = res
    return np.stack([res.results[i]["out"] for i in range(B)], axis=0)



# revision 23
# speedup vs baseline: 1.1934x; 1.0428x over previous
"""NeighbourSupport sparse-attention kernel for 8x Trainium2 NeuronCores.

Reference computation (per sample, C=256, Ck=Cv=32, H=W=128):
    k  = relu(conv1x1(x, Wk1, bk1))          # (32, H, W)
    k  = dwconv3x3(k, Wdw, bdw)              # (32, H, W), zero pad
    k  = conv1x1(k, Wk3, bk3)                # (9, H, W)
    w  = softmax(k, axis=0)                  # (9, H, W)
    v  = conv1x1(x, Wv, bv)                  # (32, H, W)
    y[c,p] = sum_j w[j,p] * v[c, p+off_j]    # 3x3 neighbourhood, zero pad
    out = x + conv1x1(y, Wo, bo)             # (256, H, W)

Sharding: pure data parallel, one sample per core (B=8, 8 cores).

Per-core layout (v2, the default): channels on SBUF partitions, pixels on
the free dim, with 4 row-groups x 32 channels packed into the 128
partitions so elementwise ops use all DVE/Pool lanes.  The image is
processed in T=8 row-tiles of R=16 rows; k1/v live in [128, 2, 6, 130]
tiles (6 local rows per group: 4 interior + 1-row halo duplicated
between neighbouring groups/tiles by SBUF->SBUF DMA, plus zero pad
columns) so the depthwise conv and the 3x3 neighbourhood aggregation are
pure shifted-view elementwise ops.  The K=256 input convs run as fp32
matmuls packed 4-per-PSUM-bank via tile_position col groups; everything
downstream (dwconv, logits, softmax, aggregation, out conv operands)
is bf16 (full-rate matmuls, DVE 2x), while the residual add stays fp32
exact.  Softmax over the 9 neighbours: ones-matmul partition reduction,
DVE reciprocal, and one-hot matmuls to broadcast per-pixel weights to
the 32 value channels.  x is streamed from HBM exactly once, out
written once (~33.6 MB/core total HBM traffic).
"""

import numpy as np

C = 256
CK = 32
H = 128
W = 128
R = 16           # rows per tile
T = H // R       # 8 tiles
NCH = 4          # chunks per tile
CR = R // NCH    # 4 rows per chunk
N = CR * W       # 512 pixels per chunk
WP = W + 2       # padded row length (130)

MM_DTYPE = "float32r"   # matmul input dtype view ("float32r" or "float32")

TAPS = [(dy, dx) for dy in (-1, 0, 1) for dx in (-1, 0, 1)]  # jj = 3(dy+1)+(dx+1)


def build_nc(mm_dtype=MM_DTYPE):
    from concourse import bacc
    import concourse.mybir as mybir
    import concourse.tile as tile

    dt = mybir.dt
    f32 = dt.float32
    mmdt = getattr(dt, mm_dtype)
    Alu = mybir.AluOpType
    Act = mybir.ActivationFunctionType

    def mm(ap):
        return ap.bitcast(mmdt) if mm_dtype != "float32" else ap

    nc = bacc.Bacc(None, target_bir_lowering=False, debug=True)

    with tile.TileContext(nc) as tc:
        with tc.tile_pool(name="dram", bufs=1, space="DRAM") as dram:
            x_d = dram.tile([C, H, W], f32, kind="ExternalInput", name="x", uniquify=False)
            out_d = dram.tile([C, H, W], f32, kind="ExternalOutput", name="out", uniquify=False)
            wk1_d = dram.tile([2, 128, CK], f32, kind="ExternalInput", name="wk1T", uniquify=False)
            wv_d = dram.tile([2, 128, CK], f32, kind="ExternalInput", name="wvT", uniquify=False)
            wo_d = dram.tile([CK, C], f32, kind="ExternalInput", name="woT", uniquify=False)
            wk3_d = dram.tile([CK, 9], f32, kind="ExternalInput", name="wk3T", uniquify=False)
            wdw_d = dram.tile([CK, 9], f32, kind="ExternalInput", name="wdw9", uniquify=False)
            bk1_d = dram.tile([CK, 1], f32, kind="ExternalInput", name="bk1c", uniquify=False)
            bv_d = dram.tile([CK, 1], f32, kind="ExternalInput", name="bvc", uniquify=False)
            bdw_d = dram.tile([CK, 1], f32, kind="ExternalInput", name="bdwc", uniquify=False)
            bk3_d = dram.tile([9, 1], f32, kind="ExternalInput", name="bk3c", uniquify=False)
            bo_d = dram.tile([128, 2], f32, kind="ExternalInput", name="boc", uniquify=False)
            ones9_d = dram.tile([9, 1], f32, kind="ExternalInput", name="ones9", uniquify=False)
            ones19_d = dram.tile([1, 9], f32, kind="ExternalInput", name="ones19", uniquify=False)
            bcast_d = dram.tile([9, 288], f32, kind="ExternalInput", name="bcast", uniquify=False)

            with (
                tc.tile_pool(name="consts", bufs=1) as cpool,
                tc.tile_pool(name="xp", bufs=3) as xpool,
                tc.tile_pool(name="kvp", bufs=3) as kvpool,
                tc.tile_pool(name="scr", bufs=3) as scpool,
                tc.tile_pool(name="outp", bufs=3) as outpool,
                tc.tile_pool(name="ps_conv", bufs=2, space="PSUM") as psA,
                tc.tile_pool(name="ps_small", bufs=2, space="PSUM") as psS,
                tc.tile_pool(name="ps_wb", bufs=2, space="PSUM") as psW,
                tc.tile_pool(name="ps_out", bufs=2, space="PSUM") as psO,
            ):
                # ---- constants into SBUF ----
                wk1s = cpool.tile([128, 2, CK], f32, name="wk1s")
                wvs = cpool.tile([128, 2, CK], f32, name="wvs")
                for h in range(2):
                    nc.sync.dma_start(out=wk1s[:, h, :], in_=wk1_d[h])
                    nc.sync.dma_start(out=wvs[:, h, :], in_=wv_d[h])
                wos = cpool.tile([CK, C], f32, name="wos")
                nc.sync.dma_start(out=wos[:], in_=wo_d[:])
                wk3s = cpool.tile([CK, 9], f32, name="wk3s")
                nc.sync.dma_start(out=wk3s[:], in_=wk3_d[:])
                wdws = cpool.tile([CK, 9], f32, name="wdws")
                nc.sync.dma_start(out=wdws[:], in_=wdw_d[:])
                bk1s = cpool.tile([CK, 1], f32, name="bk1s")
                nc.sync.dma_start(out=bk1s[:], in_=bk1_d[:])
                bvs = cpool.tile([CK, 1], f32, name="bvs")
                nc.sync.dma_start(out=bvs[:], in_=bv_d[:])
                bdws = cpool.tile([CK, 1], f32, name="bdws")
                nc.sync.dma_start(out=bdws[:], in_=bdw_d[:])
                bk3s = cpool.tile([9, 1], f32, name="bk3s")
                nc.sync.dma_start(out=bk3s[:], in_=bk3_d[:])
                bos = cpool.tile([128, 2], f32, name="bos")
                nc.sync.dma_start(out=bos[:], in_=bo_d[:])
                ones9s = cpool.tile([9, 1], f32, name="ones9s")
                nc.sync.dma_start(out=ones9s[:], in_=ones9_d[:])
                ones19s = cpool.tile([1, 9], f32, name="ones19s")
                nc.sync.dma_start(out=ones19s[:], in_=ones19_d[:])
                bcasts = cpool.tile([9, 288], f32, name="bcasts")
                nc.sync.dma_start(out=bcasts[:], in_=bcast_d[:])

                xt = [None] * T    # (x_lo, x_hi) per tile
                kvt = [None] * T   # (k1, v) per tile

                def emit_A(t):
                    x_lo = xpool.tile([128, R, W], f32, name="x_lo")
                    x_hi = xpool.tile([128, R, W], f32, name="x_hi")
                    nc.sync.dma_start(out=x_lo[:], in_=x_d[0:128, t * R:(t + 1) * R, :])
                    nc.sync.dma_start(out=x_hi[:], in_=x_d[128:256, t * R:(t + 1) * R, :])
                    k1 = kvpool.tile([CK, R + 2, WP], f32, name="k1")
                    v = kvpool.tile([CK, R + 2, WP], f32, name="v")
                    # zero the left/right pad columns
                    nc.gpsimd.memset(k1[:, :, 0:1], 0.0)
                    nc.gpsimd.memset(k1[:, :, WP - 1:WP], 0.0)
                    nc.gpsimd.memset(v[:, :, 0:1], 0.0)
                    nc.gpsimd.memset(v[:, :, WP - 1:WP], 0.0)
                    for q in range(NCH):
                        xl = x_lo[:, q * CR:(q + 1) * CR, :]
                        xh = x_hi[:, q * CR:(q + 1) * CR, :]
                        kp = psA.tile([CK, CR, W], f32, name="kp", tag="ps_conv")
                        nc.tensor.matmul(kp[:], mm(wk1s[:, 0, :]), mm(xl), start=True, stop=False)
                        nc.tensor.matmul(kp[:], mm(wk1s[:, 1, :]), mm(xh), start=False, stop=True)
                        nc.scalar.activation(k1[:, 1 + q * CR:1 + (q + 1) * CR, 1:1 + W],
                                             kp[:], Act.Relu, bias=bk1s[:, 0:1])
                        vp = psA.tile([CK, CR, W], f32, name="vp", tag="ps_conv")
                        nc.tensor.matmul(vp[:], mm(wvs[:, 0, :]), mm(xl), start=True, stop=False)
                        nc.tensor.matmul(vp[:], mm(wvs[:, 1, :]), mm(xh), start=False, stop=True)
                        nc.scalar.activation(v[:, 1 + q * CR:1 + (q + 1) * CR, 1:1 + W],
                                             vp[:], Act.Identity, bias=bvs[:, 0:1])
                    xt[t] = (x_lo, x_hi)
                    kvt[t] = (k1, v)

                def emit_B(u):
                    k1, v = kvt[u]
                    # fill halo rows (row 0 = image row u*R-1, row R+1 = image row u*R+R)
                    if u > 0:
                        pk1, pv = kvt[u - 1]
                        nc.scalar.copy(k1[:, 0, :], pk1[:, R, :])
                        nc.scalar.copy(v[:, 0, :], pv[:, R, :])
                    else:
                        nc.gpsimd.memset(k1[:, 0, :], 0.0)
                        nc.gpsimd.memset(v[:, 0, :], 0.0)
                    if u < T - 1:
                        nk1, nv = kvt[u + 1]
                        nc.scalar.copy(k1[:, R + 1, :], nk1[:, 1, :])
                        nc.scalar.copy(v[:, R + 1, :], nv[:, 1, :])
                    else:
                        nc.gpsimd.memset(k1[:, R + 1, :], 0.0)
                        nc.gpsimd.memset(v[:, R + 1, :], 0.0)

                    x_lo, x_hi = xt[u]
                    out_lo = outpool.tile([128, R, W], f32, name="out_lo")
                    out_hi = outpool.tile([128, R, W], f32, name="out_hi")

                    for q in range(NCH):
                        r0 = 1 + q * CR  # local row of first output row of chunk

                        # depthwise 3x3 on k1 -> k2
                        k2 = scpool.tile([CK, CR, W], f32, name="k2")
                        for jj, (dy, dx) in enumerate(TAPS):
                            kv_view = k1[:, r0 + dy:r0 + dy + CR, 1 + dx:1 + dx + W]
                            if jj == 0:
                                nc.vector.tensor_scalar(k2[:], kv_view,
                                                        wdws[:, 0:1], bdws[:, 0:1],
                                                        Alu.mult, Alu.add)
                            else:
                                nc.vector.scalar_tensor_tensor(k2[:], kv_view,
                                                               wdws[:, jj:jj + 1], k2[:],
                                                               Alu.mult, Alu.add)

                        # logits -> exp -> sum -> reciprocal -> normalized w9
                        lg = psS.tile([9, CR, W], f32, name="lg", tag="ps_small")
                        nc.tensor.matmul(lg[:], mm(wk3s[:]), mm(k2[:]), start=True, stop=True)
                        e = scpool.tile([9, CR, W], f32, name="e")
                        nc.scalar.activation(e[:], lg[:], Act.Exp, bias=bk3s[:, 0:1])
                        S = psS.tile([1, CR, W], f32, name="S", tag="ps_small")
                        nc.tensor.matmul(S[:], mm(ones9s[:]), mm(e[:]), start=True, stop=True)
                        rc = scpool.tile([1, CR, W], f32, name="rc")
                        nc.vector.reciprocal(rc[:], S[:])
                        r9 = psS.tile([9, CR, W], f32, name="r9", tag="ps_small")
                        nc.tensor.matmul(r9[:], mm(ones19s[:]), mm(rc[:]), start=True, stop=True)
                        w9 = scpool.tile([9, CR, W], f32, name="w9")
                        nc.vector.tensor_tensor(w9[:], e[:], r9[:], Alu.mult)

                        # aggregation: y[c,p] = sum_j w9[j,p] * v[c, p+off_j]
                        y_acc = scpool.tile([CK, CR, W], f32, name="y_acc")
                        for g in range(3):
                            wb = psW.tile([96, CR, W], f32, name="wb", tag="ps_wb")
                            nc.tensor.matmul(wb[:], mm(bcasts[:, 96 * g:96 * (g + 1)]),
                                             mm(w9[:]), start=True, stop=True)
                            for a in range(3):
                                jj = 3 * g + a
                                dy, dx = TAPS[jj]
                                v_view = v[:, r0 + dy:r0 + dy + CR, 1 + dx:1 + dx + W]
                                wbs = wb[32 * a:32 * (a + 1), :, :]
                                if jj == 0:
                                    nc.vector.tensor_tensor(y_acc[:], wbs, v_view, Alu.mult)
                                else:
                                    pr = scpool.tile([CK, CR, W], f32, name="pr")
                                    nc.vector.tensor_tensor(pr[:], wbs, v_view, Alu.mult)
                                    nc.gpsimd.tensor_tensor(y_acc[:], y_acc[:], pr[:], Alu.add)

                        # out conv + bias + residual
                        op_lo = psO.tile([128, CR, W], f32, name="op_lo", tag="ps_out")
                        nc.tensor.matmul(op_lo[:], mm(wos[:, 0:128]), mm(y_acc[:]),
                                         start=True, stop=True)
                        nc.vector.scalar_tensor_tensor(out_lo[:, q * CR:(q + 1) * CR, :],
                                                       op_lo[:], bos[:, 0:1],
                                                       x_lo[:, q * CR:(q + 1) * CR, :],
                                                       Alu.add, Alu.add)
                        op_hi = psO.tile([128, CR, W], f32, name="op_hi", tag="ps_out")
                        nc.tensor.matmul(op_hi[:], mm(wos[:, 128:256]), mm(y_acc[:]),
                                         start=True, stop=True)
                        nc.vector.scalar_tensor_tensor(out_hi[:, q * CR:(q + 1) * CR, :],
                                                       op_hi[:], bos[:, 1:2],
                                                       x_hi[:, q * CR:(q + 1) * CR, :],
                                                       Alu.add, Alu.add)

                    nc.sync.dma_start(out=out_d[0:128, u * R:(u + 1) * R, :], in_=out_lo[:])
                    nc.sync.dma_start(out=out_d[128:256, u * R:(u + 1) * R, :], in_=out_hi[:])

                emit_A(0)
                emit_A(1)
                for t in range(2, T):
                    emit_A(t)
                    emit_B(t - 2)
                emit_B(T - 2)
                emit_B(T - 1)

    nc.compile()
    return nc


def build_nc_v2(mm_dtype=MM_DTYPE, reps=1):
    """Partition-packed variant: 4 row-groups x 32 channels = 128 partitions.

    Each 16-row tile is processed as 4 groups of 4 rows; group a's
    channel-c data lives on partition 32a+c.  Matmuls use tile_position
    col/row groups so the 4 per-group matmuls pack into the PE array and
    one PSUM bank; elementwise ops run on all 128 partitions (4x fewer
    DVE/Pool ops than the unpacked variant).  k1p/vp tiles hold 6 local
    rows per group (1-row halo duplicated between neighbouring groups by
    SBUF->SBUF DMA, cross-tile for group 0/3 edges).
    """
    from concourse import bacc
    import concourse.mybir as mybir
    import concourse.tile as tile

    dt = mybir.dt
    f32 = dt.float32
    bf16 = dt.bfloat16
    Alu = mybir.AluOpType
    Act = mybir.ActivationFunctionType

    nc = bacc.Bacc(None, target_bir_lowering=False, debug=True)

    with tile.TileContext(nc) as tc:
        with tc.tile_pool(name="dram", bufs=1, space="DRAM") as dram:
            x_d = dram.tile([C, H, W], f32, kind="ExternalInput", name="x", uniquify=False)
            out_d = dram.tile([C, H, W], f32, kind="ExternalOutput", name="out", uniquify=False)
            wk1_d = dram.tile([2, 128, CK], f32, kind="ExternalInput", name="wk1T", uniquify=False)
            wv_d = dram.tile([2, 128, CK], f32, kind="ExternalInput", name="wvT", uniquify=False)
            wop_d = dram.tile([128, C], bf16, kind="ExternalInput", name="wop", uniquify=False)
            wk3p_d = dram.tile([128, 9], bf16, kind="ExternalInput", name="wk3p", uniquify=False)
            wdwp_d = dram.tile([128, 9], f32, kind="ExternalInput", name="wdwp", uniquify=False)
            bk1p_d = dram.tile([128, 1], f32, kind="ExternalInput", name="bk1p", uniquify=False)
            bvp_d = dram.tile([128, 1], f32, kind="ExternalInput", name="bvp", uniquify=False)
            bdwp_d = dram.tile([128, 1], f32, kind="ExternalInput", name="bdwp", uniquify=False)
            bk3p_d = dram.tile([128, 1], f32, kind="ExternalInput", name="bk3p", uniquify=False)
            bo_d = dram.tile([128, 2], f32, kind="ExternalInput", name="boc", uniquify=False)
            ones_d = dram.tile([128, 32], bf16, kind="ExternalInput", name="ones32", uniquify=False)
            bc2_d = dram.tile([128, 288], bf16, kind="ExternalInput", name="bc2", uniquify=False)

            with (
                tc.tile_pool(name="consts", bufs=1) as cpool,
                tc.tile_pool(name="xp", bufs=3) as xpool,
                tc.tile_pool(name="kvp", bufs=3) as kvpool,
                tc.tile_pool(name="scr", bufs=3) as scpool,
                tc.tile_pool(name="outp", bufs=3) as outpool,
                tc.tile_pool(name="ps_conv", bufs=2, space="PSUM") as psA,
                tc.tile_pool(name="ps_small", bufs=2, space="PSUM") as psS,
                tc.tile_pool(name="ps_wb", bufs=2, space="PSUM") as psW,
                tc.tile_pool(name="ps_out", bufs=2, space="PSUM") as psO,
            ):
                wk1s = cpool.tile([128, 2, CK], f32, name="wk1s")
                wvs = cpool.tile([128, 2, CK], f32, name="wvs")
                for h in range(2):
                    nc.sync.dma_start(out=wk1s[:, h, :], in_=wk1_d[h])
                    nc.sync.dma_start(out=wvs[:, h, :], in_=wv_d[h])
                wops = cpool.tile([128, C], bf16, name="wops")
                nc.sync.dma_start(out=wops[:], in_=wop_d[:])
                wk3s = cpool.tile([128, 9], bf16, name="wk3s")
                nc.sync.dma_start(out=wk3s[:], in_=wk3p_d[:])
                wdws = cpool.tile([128, 9], f32, name="wdws")
                nc.sync.dma_start(out=wdws[:], in_=wdwp_d[:])
                bk1s = cpool.tile([128, 1], f32, name="bk1s")
                nc.sync.dma_start(out=bk1s[:], in_=bk1p_d[:])
                bvs = cpool.tile([128, 1], f32, name="bvs")
                nc.sync.dma_start(out=bvs[:], in_=bvp_d[:])
                bdws = cpool.tile([128, 1], f32, name="bdws")
                nc.sync.dma_start(out=bdws[:], in_=bdwp_d[:])
                bk3s = cpool.tile([128, 1], f32, name="bk3s")
                nc.sync.dma_start(out=bk3s[:], in_=bk3p_d[:])
                bos = cpool.tile([128, 2], f32, name="bos")
                nc.sync.dma_start(out=bos[:], in_=bo_d[:])
                oness = cpool.tile([128, 32], bf16, name="oness")
                nc.sync.dma_start(out=oness[:], in_=ones_d[:])
                bc2s = cpool.tile([128, 288], bf16, name="bc2s")
                nc.sync.dma_start(out=bc2s[:], in_=bc2_d[:])

                xt = [None] * T
                kvt = [None] * T

                def emit_A(t):
                    x_lo = xpool.tile([128, R, W], f32, name="x_lo")
                    x_hi = xpool.tile([128, R, W], f32, name="x_hi")
                    nc.sync.dma_start(out=x_lo[:], in_=x_d[0:128, t * R:(t + 1) * R, :])
                    nc.sync.dma_start(out=x_hi[:], in_=x_d[128:256, t * R:(t + 1) * R, :])
                    kv = kvpool.tile([128, 2, 6, WP], bf16, name="kv")
                    k1p = kv[:, 0]
                    vp = kv[:, 1]
                    nc.gpsimd.memset(kv[:, :, :, 0:1], 0.0)
                    nc.gpsimd.memset(kv[:, :, :, WP - 1:WP], 0.0)
                    kp = psA.tile([128, CR, W], f32, name="kp", tag="ps_conv")
                    vpp = psA.tile([128, CR, W], f32, name="vpp", tag="ps_conv")
                    for a in range(4):
                        xl = x_lo[:, 4 * a:4 * a + 4, :]
                        xh = x_hi[:, 4 * a:4 * a + 4, :]
                        po = kp[32 * a:32 * (a + 1), :, :]
                        nc.tensor.matmul(po, wk1s[:, 0, :], xl,
                                         start=True, stop=False, tile_position=(0, 32 * a))
                        nc.tensor.matmul(po, wk1s[:, 1, :], xh,
                                         start=False, stop=True, tile_position=(0, 32 * a))
                        po = vpp[32 * a:32 * (a + 1), :, :]
                        nc.tensor.matmul(po, wvs[:, 0, :], xl,
                                         start=True, stop=False, tile_position=(0, 32 * a))
                        nc.tensor.matmul(po, wvs[:, 1, :], xh,
                                         start=False, stop=True, tile_position=(0, 32 * a))
                    nc.scalar.activation(k1p[:, 1:5, 1:1 + W], kp[:], Act.Relu,
                                         bias=bk1s[:, 0:1])
                    nc.scalar.activation(vp[:, 1:5, 1:1 + W], vpp[:], Act.Identity,
                                         bias=bvs[:, 0:1])
                    # duplicate halo rows between neighbouring groups (intra-tile)
                    nc.sync.dma_start(out=kv[32:128, :, 0, :], in_=kv[0:96, :, 4, :])
                    nc.sync.dma_start(out=kv[0:96, :, 5, :], in_=kv[32:128, :, 1, :])
                    xt[t] = (x_lo, x_hi)
                    kvt[t] = kv

                def emit_B(u):
                    kv = kvt[u]
                    k1p = kv[:, 0]
                    vp = kv[:, 1]
                    # cross-tile halo rows for group 0 (top) and group 3 (bottom)
                    if u > 0:
                        nc.sync.dma_start(out=kv[0:32, :, 0, :],
                                          in_=kvt[u - 1][96:128, :, 4, :])
                    else:
                        nc.gpsimd.memset(kv[0:32, :, 0, :], 0.0)
                    if u < T - 1:
                        nc.sync.dma_start(out=kv[96:128, :, 5, :],
                                          in_=kvt[u + 1][0:32, :, 1, :])
                    else:
                        nc.gpsimd.memset(kv[96:128, :, 5, :], 0.0)

                    x_lo, x_hi = xt[u]
                    out_lo = outpool.tile([128, R, W], f32, name="out_lo")
                    out_hi = outpool.tile([128, R, W], f32, name="out_hi")

                    def kview(tt, jj):
                        dy, dx = TAPS[jj]
                        return tt[:, 1 + dy:5 + dy, 1 + dx:1 + dx + W]

                    # depthwise 3x3: 9 taps chained on DVE (bf16, 2x mode)
                    k2m = scpool.tile([128, CR, W], bf16, name="k2m")
                    with nc.allow_low_precision(reason="bf16 dwconv accumulation"):
                        nc.vector.tensor_scalar(k2m[:], kview(k1p, 0), wdws[:, 0:1],
                                                bdws[:, 0:1], Alu.mult, Alu.add)
                        for jj in range(1, 9):
                            nc.vector.scalar_tensor_tensor(k2m[:], kview(k1p, jj),
                                                           wdws[:, jj:jj + 1], k2m[:],
                                                           Alu.mult, Alu.add)

                    # logits / exp / sum / recip per group (tile-packed matmuls)
                    lg = psS.tile([128, CR, W], f32, name="lg", tag="ps_small")
                    for a in range(4):
                        nc.tensor.matmul(lg[32 * a:32 * a + 9, :, :],
                                         wk3s[32 * a:32 * (a + 1), :],
                                         k2m[32 * a:32 * (a + 1), :, :],
                                         start=True, stop=True,
                                         tile_position=(32 * a, 32 * a))
                    e = scpool.tile([128, CR, W], bf16, name="e")
                    for a in range(4):
                        nc.scalar.activation(e[32 * a:32 * a + 9, :, :],
                                             lg[32 * a:32 * a + 9, :, :], Act.Exp,
                                             bias=bk3s[32 * a:32 * a + 9, 0:1])
                    S = psS.tile([128, CR, W], f32, name="S", tag="ps_small")
                    for a in range(4):
                        nc.tensor.matmul(S[32 * a:32 * a + 1, :, :],
                                         oness[32 * a:32 * a + 9, 0:1],
                                         e[32 * a:32 * a + 9, :, :],
                                         start=True, stop=True,
                                         tile_position=(32 * a, 32 * a))
                    rc = scpool.tile([128, CR, W], bf16, name="rc")
                    with nc.allow_low_precision(reason="bf16 softmax recip"):
                        for a in range(4):
                            nc.vector.reciprocal(rc[32 * a:32 * a + 1, :, :],
                                                 S[32 * a:32 * a + 1, :, :])
                    rb = psS.tile([128, CR, W], f32, name="rb", tag="ps_small")
                    for a in range(4):
                        nc.tensor.matmul(rb[32 * a:32 * (a + 1), :, :],
                                         oness[32 * a:32 * a + 1, 0:32],
                                         rc[32 * a:32 * a + 1, :, :],
                                         start=True, stop=True,
                                         tile_position=(32 * a, 32 * a))
                    # aggregation with unnormalized weights, normalize at the end;
                    # products on DVE, pairwise tree-adds on Pool
                    prods = []
                    for jj in range(9):
                        wb = psW.tile([128, CR, W], f32, name="wb", tag="ps_wb")
                        for a in range(4):
                            nc.tensor.matmul(wb[32 * a:32 * (a + 1), :, :],
                                             bc2s[32 * a:32 * a + 9, 32 * jj:32 * (jj + 1)],
                                             e[32 * a:32 * a + 9, :, :],
                                             start=True, stop=True,
                                             tile_position=(32 * a, 32 * a))
                        vv = kview(vp, jj)
                        pr = scpool.tile([128, CR, W], f32, name="pr", bufs=6)
                        nc.vector.tensor_tensor(pr[:], wb[:], vv, Alu.mult)
                        prods.append(pr)
                        if jj % 2 == 1:   # fold pairs as they arrive
                            nc.gpsimd.tensor_tensor(prods[jj - 1][:], prods[jj - 1][:],
                                                    prods[jj][:], Alu.add)
                    s0, s1, s2, s3, p8 = prods[0], prods[2], prods[4], prods[6], prods[8]
                    nc.gpsimd.tensor_tensor(s3[:], s3[:], p8[:], Alu.add)
                    nc.gpsimd.tensor_tensor(s0[:], s0[:], s1[:], Alu.add)
                    nc.gpsimd.tensor_tensor(s2[:], s2[:], s3[:], Alu.add)
                    nc.gpsimd.tensor_tensor(s0[:], s0[:], s2[:], Alu.add)
                    y_bf = scpool.tile([128, CR, W], bf16, name="y_bf")
                    nc.vector.tensor_tensor(y_bf[:], s0[:], rb[:], Alu.mult)

                    # out conv + bias via ACT, residual add on Pool
                    for half, (xh, outh) in enumerate(((x_lo, out_lo), (x_hi, out_hi))):
                        for a in range(4):
                            op = psO.tile([128, CR, W], f32, name="op", tag="ps_out")
                            nc.tensor.matmul(op[:],
                                             wops[32 * a:32 * (a + 1), 128 * half:128 * (half + 1)],
                                             y_bf[32 * a:32 * (a + 1), :, :],
                                             start=True, stop=True,
                                             tile_position=(32 * a, 0))
                            ov = outh[:, 4 * a:4 * a + 4, :]
                            nc.scalar.activation(ov, op[:], Act.Identity,
                                                 bias=bos[:, half:half + 1])
                            nc.gpsimd.tensor_tensor(ov, ov, xh[:, 4 * a:4 * a + 4, :],
                                                    Alu.add)

                    nc.sync.dma_start(out=out_d[0:128, u * R:(u + 1) * R, :], in_=out_lo[:])
                    nc.sync.dma_start(out=out_d[128:256, u * R:(u + 1) * R, :], in_=out_hi[:])

                def emit_all():
                    for i in range(T):
                        xt[i] = None
                        kvt[i] = None
                    emit_A(0)
                    emit_A(1)
                    for t in range(2, T):
                        emit_A(t)
                        emit_B(t - 2)
                    emit_B(T - 2)
                    emit_B(T - 1)

                if reps > 1:
                    with tc.For_i(0, reps, 1):
                        emit_all()
                else:
                    emit_all()

    nc.compile()
    return nc


def build_nc_v3():
    """v3: blockdiag weights instead of tile_position packing, f32r input
    convs, dwconv+Wk3 fused into 9 accumulating matmuls, packed exp,
    reciprocal_approx_fast, residual via identity matmul into PSUM.

    Layout identical to v2: 4 row-groups x 32 channels on 128 partitions,
    T=8 tiles of R=16 rows, kv tiles [128, 2, 6, 130] bf16 with halo rows.
    Group-a quantities that are per-pixel scalars live at partitions 9a+o
    (logits/e, o<9) or a (sums), via block-diagonal lhsT weights.
    """
    from concourse import bacc
    import concourse.mybir as mybir
    import concourse.tile as tile

    dt = mybir.dt
    f32 = dt.float32
    f32r = dt.float32r
    bf16 = dt.bfloat16
    f8 = dt.float8e4
    DR = mybir.MatmulPerfMode.DoubleRow
    Alu = mybir.AluOpType
    Act = mybir.ActivationFunctionType

    def r(ap):
        return ap.bitcast(f32r)

    nc = bacc.Bacc(None, target_bir_lowering=False, debug=True)

    with tile.TileContext(nc) as tc:
        with tc.tile_pool(name="dram", bufs=1, space="DRAM") as dram:
            x_d = dram.tile([C, H, W], f32, kind="ExternalInput", name="x", uniquify=False)
            out_d = dram.tile([C, H, W], f32, kind="ExternalOutput", name="out", uniquify=False)
            wk1_d = dram.tile([2, 128, CK], bf16, kind="ExternalInput", name="wk1T", uniquify=False)
            wv_d = dram.tile([2, 128, CK], bf16, kind="ExternalInput", name="wvT", uniquify=False)
            wop_d = dram.tile([128, C], bf16, kind="ExternalInput", name="wop", uniquify=False)
            mf_d = dram.tile([128, 9, 36], bf16, kind="ExternalInput", name="mf", uniquify=False)
            wbm_d = dram.tile([36, 9, 128], bf16, kind="ExternalInput", name="wbm", uniquify=False)
            sones_d = dram.tile([36, 4], bf16, kind="ExternalInput", name="sones", uniquify=False)
            bk1p_d = dram.tile([128, 1], f32, kind="ExternalInput", name="bk1p", uniquify=False)
            bvp_d = dram.tile([128, 1], f32, kind="ExternalInput", name="bvp", uniquify=False)
            bk3f_d = dram.tile([36, 1], f32, kind="ExternalInput", name="bk3f", uniquify=False)
            bo_d = dram.tile([128, 2], f32, kind="ExternalInput", name="boc", uniquify=False)

            with (
                tc.tile_pool(name="consts", bufs=1) as cpool,
                tc.tile_pool(name="xbp", bufs=5) as xbpool,
                tc.tile_pool(name="kvp", bufs=4) as kvpool,
                tc.tile_pool(name="scr", bufs=3) as scpool,
                tc.tile_pool(name="outp", bufs=3) as outpool,
                tc.tile_pool(name="ps_conv", bufs=2, space="PSUM") as psA,
                tc.tile_pool(name="ps_small", bufs=2, space="PSUM") as psS,
                tc.tile_pool(name="ps_wb", bufs=2, space="PSUM") as psW,
                tc.tile_pool(name="ps_out", bufs=2, space="PSUM") as psO,

            ):
                wk1s = cpool.tile([128, 2, CK], bf16, name="wk1s")
                wvs = cpool.tile([128, 2, CK], bf16, name="wvs")
                for h in range(2):
                    nc.sync.dma_start(out=wk1s[:, h, :], in_=wk1_d[h])
                    nc.sync.dma_start(out=wvs[:, h, :], in_=wv_d[h])
                wops = cpool.tile([128, C], bf16, name="wops")
                nc.sync.dma_start(out=wops[:], in_=wop_d[:])
                mfs = cpool.tile([128, 9, 36], bf16, name="mfs")
                nc.sync.dma_start(out=mfs[:], in_=mf_d[:])
                wbms = cpool.tile([36, 9, 128], bf16, name="wbms")
                nc.sync.dma_start(out=wbms[:], in_=wbm_d[:])
                soness = cpool.tile([36, 4], bf16, name="soness")
                nc.sync.dma_start(out=soness[:], in_=sones_d[:])
                bk1s = cpool.tile([128, 1], f32, name="bk1s")
                nc.sync.dma_start(out=bk1s[:], in_=bk1p_d[:])
                bvs = cpool.tile([128, 1], f32, name="bvs")
                nc.sync.dma_start(out=bvs[:], in_=bvp_d[:])
                bk3s = cpool.tile([36, 1], f32, name="bk3s")
                nc.sync.dma_start(out=bk3s[:], in_=bk3f_d[:])
                bos = cpool.tile([128, 2], f32, name="bos")
                nc.sync.dma_start(out=bos[:], in_=bo_d[:])

                xt = [None] * T
                kvt = [None] * T
                et = [None] * T

                def emit_A(t):
                    xb_lo = xbpool.tile([128, R, W], bf16, name="xb_lo")
                    xb_hi = xbpool.tile([128, R, W], bf16, name="xb_hi")
                    nc.gpsimd.dma_start(out=xb_lo[:], in_=x_d[0:128, t * R:(t + 1) * R, :])
                    nc.gpsimd.dma_start(out=xb_hi[:], in_=x_d[128:256, t * R:(t + 1) * R, :])
                    kv = kvpool.tile([128, 2, 6, WP], bf16, name="kv")
                    k1p = kv[:, 0]
                    vp = kv[:, 1]
                    nc.gpsimd.memset(kv[:, :, :, 0:1], 0.0)
                    nc.gpsimd.memset(kv[:, :, :, WP - 1:WP], 0.0)
                    kp = psA.tile([128, CR, W], f32, name="kp", tag="ps_conv")
                    vpp = psA.tile([128, CR, W], f32, name="vpp", tag="ps_conv")
                    for a in range(4):
                        xl = xb_lo[:, 4 * a:4 * a + 4, :]
                        xh = xb_hi[:, 4 * a:4 * a + 4, :]
                        po = kp[32 * a:32 * (a + 1), :, :]
                        nc.tensor.matmul(po, wk1s[:, 0, :], xl,
                                         start=True, stop=False, tile_position=(0, 32 * a))
                        nc.tensor.matmul(po, wk1s[:, 1, :], xh,
                                         start=False, stop=True, tile_position=(0, 32 * a))
                        po = vpp[32 * a:32 * (a + 1), :, :]
                        nc.tensor.matmul(po, wvs[:, 0, :], xl,
                                         start=True, stop=False, tile_position=(0, 32 * a))
                        nc.tensor.matmul(po, wvs[:, 1, :], xh,
                                         start=False, stop=True, tile_position=(0, 32 * a))
                    nc.scalar.activation(k1p[:, 1:5, 1:1 + W], kp[:], Act.Relu,
                                         bias=bk1s[:, 0:1])
                    nc.scalar.activation(vp[:, 1:5, 1:1 + W], vpp[:], Act.Identity,
                                         bias=bvs[:, 0:1])
                    nc.gpsimd.dma_start(out=kv[32:128, :, 0, :], in_=kv[0:96, :, 4, :])
                    nc.gpsimd.dma_start(out=kv[0:96, :, 5, :], in_=kv[32:128, :, 1, :])
                    xt[t] = (xb_lo, xb_hi)
                    kvt[t] = kv

                def emit_B1(u):
                    """Softmax pipe: halos, fused dwconv+Wk3 logits, exp,
                    sum, reciprocal, reciprocal broadcast."""
                    kv = kvt[u]
                    k1p = kv[:, 0]
                    if u > 0:
                        nc.gpsimd.dma_start(out=kv[0:32, :, 0, :],
                                            in_=kvt[u - 1][96:128, :, 4, :])
                    else:
                        nc.gpsimd.memset(kv[0:32, :, 0, :], 0.0)
                    if u < T - 1:
                        nc.gpsimd.dma_start(out=kv[96:128, :, 5, :],
                                            in_=kvt[u + 1][0:32, :, 1, :])
                    else:
                        nc.gpsimd.memset(kv[96:128, :, 5, :], 0.0)

                    def kview(tt, jj):
                        dy, dx = TAPS[jj]
                        return tt[:, 1 + dy:5 + dy, 1 + dx:1 + dx + W]

                    lg = psS.tile([36, CR, W], f32, name="lg", tag="ps_small")
                    for jj in range(9):
                        nc.tensor.matmul(lg[:], mfs[:, jj, :], kview(k1p, jj),
                                         start=(jj == 0), stop=(jj == 8))
                    e = scpool.tile([36, CR, W], bf16, name="e")
                    with nc.allow_low_precision(reason="bf16 softmax weights"):
                        nc.scalar.activation(e[:], lg[:], Act.Exp, bias=bk3s[:, 0:1])
                    S = psS.tile([4, CR, W], f32, name="S", tag="ps_small")
                    nc.tensor.matmul(S[:], soness[:], e[:], start=True, stop=True)
                    rc = scpool.tile([4, CR, W], f32, name="rc")
                    with nc.allow_low_precision(reason="approx reciprocal"):
                        nc.vector.reciprocal_approx_fast(out=rc[:], in_=S[:])
                    rc_bf = scpool.tile([4, CR, W], bf16, name="rc_bf")
                    with nc.allow_low_precision(reason="bf16 softmax recip"):
                        nc.scalar.copy(rc_bf[:], rc[:])
                    rb_sb = scpool.tile([128, CR, W], bf16, name="rb_sb")
                    nc.gpsimd.dma_start(
                        out=rb_sb[:],
                        in_=rc_bf[:].unsqueeze(1).broadcast_to((4, 32, CR, W)))
                    et[u] = (e, rb_sb)

                def emit_B2(u):
                    """Aggregation + out conv + residual + store."""
                    kv = kvt[u]
                    vp = kv[:, 1]
                    e, rb = et[u]
                    x_lo, x_hi = xt[u]
                    out_lo = outpool.tile([128, R, W], f32, name="out_lo")
                    out_hi = outpool.tile([128, R, W], f32, name="out_hi")

                    def kview(tt, jj):
                        dy, dx = TAPS[jj]
                        return tt[:, 1 + dy:5 + dy, 1 + dx:1 + dx + W]

                    prods = []
                    for jj in range(9):
                        wb = psW.tile([128, CR, W], f32, name="wb", tag="ps_wb")
                        nc.tensor.matmul(wb[:], wbms[:, jj, :], e[:],
                                         start=True, stop=True)
                        vv = kview(vp, jj)
                        pr = scpool.tile([128, CR, W], bf16, name="pr", bufs=6)
                        with nc.allow_low_precision(reason="bf16 aggregation"):
                            nc.vector.tensor_tensor(pr[:], wb[:], vv, Alu.mult)
                        prods.append(pr)
                        if jj % 2 == 1:
                            nc.vector.tensor_tensor(prods[jj - 1][:], prods[jj - 1][:],
                                                    prods[jj][:], Alu.add)
                    s0, s1, s2, s3, p8 = prods[0], prods[2], prods[4], prods[6], prods[8]
                    nc.gpsimd.tensor_tensor(s3[:], s3[:], p8[:], Alu.add)
                    nc.gpsimd.tensor_tensor(s0[:], s0[:], s1[:], Alu.add)
                    nc.gpsimd.tensor_tensor(s2[:], s2[:], s3[:], Alu.add)
                    nc.gpsimd.tensor_tensor(s0[:], s0[:], s2[:], Alu.add)
                    y_bf = scpool.tile([128, CR, W], bf16, name="y_bf")
                    with nc.allow_low_precision(reason="bf16 aggregation"):
                        nc.vector.tensor_tensor(y_bf[:], s0[:], rb[:], Alu.mult)

                    for half, (xh, outh) in enumerate(((x_lo, out_lo), (x_hi, out_hi))):
                        for a in range(4):
                            op = psO.tile([128, CR, W], f32, name="op", tag="ps_out")
                            nc.tensor.matmul(op[:],
                                             wops[32 * a:32 * (a + 1), 128 * half:128 * (half + 1)],
                                             y_bf[32 * a:32 * (a + 1), :, :],
                                             start=True, stop=True,
                                             tile_position=(32 * a, 0))
                            ov = outh[:, 4 * a:4 * a + 4, :]
                            if half == 0:
                                nc.vector.scalar_tensor_tensor(ov, op[:],
                                                               bos[:, 0:1],
                                                               xh[:, 4 * a:4 * a + 4, :],
                                                               Alu.add, Alu.add)
                            else:
                                nc.scalar.activation(ov, op[:], Act.Identity,
                                                     bias=bos[:, 1:2])
                                nc.gpsimd.tensor_tensor(ov, ov,
                                                        xh[:, 4 * a:4 * a + 4, :],
                                                        Alu.add)

                    nc.scalar.dma_start(out=out_d[0:128, u * R:(u + 1) * R, :], in_=out_lo[:])
                    nc.scalar.dma_start(out=out_d[128:256, u * R:(u + 1) * R, :], in_=out_hi[:])

                emit_A(0)
                emit_A(1)
                emit_B1(0)
                emit_A(2)
                for t in range(3, T):
                    emit_A(t)
                    emit_B2(t - 3)
                    emit_B1(t - 2)
                emit_B1(T - 2)
                emit_B2(T - 3)
                emit_B1(T - 1)
                emit_B2(T - 2)
                emit_B2(T - 1)

    nc.compile()
    return nc


def make_const_inputs_v3(Wk1, bk1, Wdw, bdw, Wk3, bk3, Wv, bv, Wo, bo):
    import ml_dtypes
    f = np.float32
    bf = ml_dtypes.bfloat16
    f8 = ml_dtypes.float8_e4m3
    Wdw9 = Wdw.reshape(CK, 9).astype(f)       # [c, j]
    mf = np.zeros((128, 9, 36), f)
    for a in range(4):
        for c in range(CK):
            for j in range(9):
                # lg[9a+o] += sum_c Wk3[o,c]*Wdw[c,j] * k1[32a+c, p+delta_j]
                mf[32 * a + c, j, 9 * a:9 * a + 9] = Wk3[:, c] * Wdw9[c, j]
    bk3f = np.zeros((36, 1), f)
    for a in range(4):
        bk3f[9 * a:9 * a + 9, 0] = bk3 + Wk3 @ bdw
    sones = np.zeros((36, 4), f)
    for a in range(4):
        sones[9 * a:9 * a + 9, a] = 1.0
    wbm = np.zeros((36, 9, 128), f)
    for a in range(4):
        for j in range(9):
            wbm[9 * a + j, j, 32 * a:32 * (a + 1)] = 1.0
    return {
        "wk1T": np.ascontiguousarray(Wk1.T.reshape(2, 128, CK), f).astype(bf),
        "wvT": np.ascontiguousarray(Wv.T.reshape(2, 128, CK), f).astype(bf),
        "wop": np.ascontiguousarray(np.tile(Wo.T, (4, 1))).astype(bf),
        "mf": mf.astype(bf),
        "sones": sones.astype(bf),
        "wbm": wbm.astype(bf),
        "bk1p": np.ascontiguousarray(np.tile(bk1.reshape(CK, 1), (4, 1)), f),
        "bvp": np.ascontiguousarray(np.tile(bv.reshape(CK, 1), (4, 1)), f),
        "bk3f": bk3f,
        "boc": np.ascontiguousarray(bo.reshape(2, 128).T, f),
    }


def make_const_inputs_v2(Wk1, bk1, Wdw, bdw, Wk3, bk3, Wv, bv, Wo, bo):
    import ml_dtypes
    f = np.float32
    bf = ml_dtypes.bfloat16
    bc2 = np.zeros((128, 288), bf)
    for a in range(4):
        for j in range(9):
            bc2[32 * a + j, 32 * j:32 * (j + 1)] = 1.0
    bk3p = np.zeros((128, 1), f)
    for a in range(4):
        bk3p[32 * a:32 * a + 9, 0] = bk3
    return {
        "wk1T": np.ascontiguousarray(Wk1.T.reshape(2, 128, CK), f),
        "wvT": np.ascontiguousarray(Wv.T.reshape(2, 128, CK), f),
        "wop": np.ascontiguousarray(np.tile(Wo.T, (4, 1))).astype(bf),
        "wk3p": np.ascontiguousarray(np.tile(Wk3.T, (4, 1))).astype(bf),
        "wdwp": np.ascontiguousarray(np.tile(Wdw.reshape(CK, 9), (4, 1)), f),
        "bk1p": np.ascontiguousarray(np.tile(bk1.reshape(CK, 1), (4, 1)), f),
        "bvp": np.ascontiguousarray(np.tile(bv.reshape(CK, 1), (4, 1)), f),
        "bdwp": np.ascontiguousarray(np.tile(bdw.reshape(CK, 1), (4, 1)), f),
        "bk3p": bk3p,
        "boc": np.ascontiguousarray(bo.reshape(2, 128).T, f),
        "ones32": np.ones((128, 32), bf),
        "bc2": bc2,
    }


def make_const_inputs(Wk1, bk1, Wdw, bdw, Wk3, bk3, Wv, bv, Wo, bo):
    f = np.float32
    bcast = np.zeros((9, 288), f)
    for j in range(9):
        g, a = divmod(j, 3)
        bcast[j, 96 * g + 32 * a:96 * g + 32 * (a + 1)] = 1.0
    return {
        "wk1T": np.ascontiguousarray(Wk1.T.reshape(2, 128, CK), f),
        "wvT": np.ascontiguousarray(Wv.T.reshape(2, 128, CK), f),
        "woT": np.ascontiguousarray(Wo.T, f),
        "wk3T": np.ascontiguousarray(Wk3.T, f),
        "wdw9": np.ascontiguousarray(Wdw.reshape(CK, 9), f),
        "bk1c": np.ascontiguousarray(bk1.reshape(CK, 1), f),
        "bvc": np.ascontiguousarray(bv.reshape(CK, 1), f),
        "bdwc": np.ascontiguousarray(bdw.reshape(CK, 1), f),
        "bk3c": np.ascontiguousarray(bk3.reshape(9, 1), f),
        "boc": np.ascontiguousarray(bo.reshape(2, 128).T, f),
        "ones9": np.ones((9, 1), f),
        "ones19": np.ones((1, 9), f),
        "bcast": bcast,
    }


VERSION = 3

_NC_CACHE = {}


def build():
    if VERSION == 3:
        return build_nc_v3()
    return build_nc_v2(MM_DTYPE) if VERSION == 2 else build_nc(MM_DTYPE)


def consts(**kw):
    if VERSION == 3:
        return make_const_inputs_v3(**kw)
    fn = make_const_inputs_v2 if VERSION == 2 else make_const_inputs
    return fn(**kw)


def _get_nc():
    key = (VERSION, MM_DTYPE)
    if key not in _NC_CACHE:
        _NC_CACHE[key] = build()
    return _NC_CACHE[key]


LAST_RESULT = None


def kernel(x, Wk1, bk1, Wdw, bdw, Wk3, bk3, Wv, bv, Wo, bo):
    global LAST_RESULT
    from concourse.bass_utils import run_bass_kernel_spmd

    x = np.asarray(x, np.float32)
    B = x.shape[0]
    assert B == 8 and x.shape[1:] == (C, H, W)
    cs = consts(Wk1=np.asarray(Wk1), bk1=np.asarray(bk1), Wdw=np.asarray(Wdw),
                bdw=np.asarray(bdw), Wk3=np.asarray(Wk3), bk3=np.asarray(bk3),
                Wv=np.asarray(Wv), bv=np.asarray(bv), Wo=np.asarray(Wo),
                bo=np.asarray(bo))
    nc = _get_nc()
    in_maps = [dict(cs, x=np.ascontiguousarray(x[i])) for i in range(B)]
    res = run_bass_kernel_spmd(nc, in_maps, list(range(B)))
    LAST_RESULT = res
    return np.stack([res.results[i]["out"] for i in range(B)], axis=0)

